# revision 29
# baseline (speedup 1.0000x reference)
"""Trainium2 Bass kernel for nn_EventSequenceDurationGraphConvModel.

Self-contained: accepts FULL inputs, shards across 8 NeuronCores internally
(nodes/edges partitioned by destination node per core), runs one SPMD Bass
program, and returns the FULL [64, 16] output.

Per-core GraphConv layers aggregate via dma_gather of source rows (bf16)
followed by PSUM matmuls against one-hot selection matrices
W_sel[e, d] = ew[e] * (dst_rel[e] == d), so the segment_sum needs no
scatter. Key performance structure:
  - SWDGE descriptor dispatch is the machine bottleneck (~5ns/descriptor,
    one descriptor per gathered edge row). Gathers are spread across 4
    SWDGE queues (ucode max), which roughly halves effective dispatch
    time vs a single queue. single_packet must stay False (True crashes).
  - W_sel matrices are built on-device with TWO broadcast tensor_tensor
    mega-ops per block group (is_equal against an iota row, then in-place
    multiply by edge weights) -- cheap on DVE and no HBM traffic to
    contend with gather descriptor dispatch.
  - c1 gathers ONE combined [h2|d] table with one descriptor per edge
    instead of two, saving ~20% of all descriptors. The table is FP8
    (256B rows): halves the g2->c1 AllGather volume (the one serial
    collective bubble) and c1's gather bytes; aggregation noise is
    diluted by exact f32 root terms and the 780-node mean pool
    (measured end-to-end rel err 6.6e-4 vs 2e-2 budget).
  - dur table is bf16 padded to 128 features so d1's gather needs no
    f32 download + cast.
  - Root-term inputs stay feature-major ([F, nodes]); c1's output stage
    computes node-major y directly (swapped matmul operands, bias folded
    in via a rank-1 ones-row matmul), so no PE transposes there.
  - Host pre-masks x (x == -1.0 -> 0); the reference's post-layer mask
    ops are no-ops given relu(-1.0) == 0, so no device masking at all.
  - gather tiles are triple-buffered so descriptor generation for group
    n+2 overlaps compute of group n.

Pipeline per core (fp32 accumulation, bf16 gathers/matmuls):
  g1 -> AllGather(h1) overlapped with d1 -> g2 -> AllGather([h2|d]) -> c1
  -> pool (PSUM matmul against host-built one-hot graph membership)
  -> AllReduce -> replicated MLP head + log_softmax.
"""
import sys

import numpy as np

sys.path.insert(0, "/opt/trn_rl_repo")

from concourse import bacc, bass, mybir  # noqa: E402
import concourse.tile as tile  # noqa: E402
from concourse.masks import make_identity  # noqa: E402

F32 = mybir.dt.float32
BF16 = mybir.dt.bfloat16
F8 = mybir.dt.float8e4
I16 = mybir.dt.int16
AF = mybir.ActivationFunctionType
OP = mybir.AluOpType

NC = 8

REAL = dict(N=50000, E=800000, B=64, SHARD=6250, SHARD_PAD=6272)


# --------------------------------------------------------------------------
# Host-side sharding / preprocessing (pure index/layout work)
# --------------------------------------------------------------------------

def _gpid(node_id, cfg):
    """Real node id -> padded global id."""
    return (node_id // cfg["SHARD"]) * cfg["SHARD_PAD"] + node_id % cfg["SHARD"]


def _wrap_idx(flat_i16):
    """Flat int16 index list -> dma_gather plane [128, n/16] (16-part wrap,
    replicated across the 8 gpsimd cores)."""
    n = flat_i16.shape[0]
    assert n % 16 == 0
    return np.tile(flat_i16.reshape(n // 16, 16).T, (8, 1)).copy()


def prep_edges(edge_index, edge_attr, cfg):
    """Shard + sort + pad the edge list. Returns per-core gather planes,
    host-built W_sel planes, and uniform per-block chunk counts (CA, CB)."""
    import ml_dtypes
    n_blk = cfg["SHARD_PAD"] // 128
    half = NC * cfg["SHARD_PAD"] // 2
    src = np.asarray(edge_index[0], dtype=np.int64)
    dst = np.asarray(edge_index[1], dtype=np.int64)
    ew = np.asarray(edge_attr, dtype=np.float32)
    gsrc = _gpid(src, cfg)
    core = dst // cfg["SHARD"]
    dloc = dst % cfg["SHARD"]

    per_core = []
    ca_max = cb_max = 1
    for k in range(NC):
        sel = np.nonzero(core == k)[0]
        order = sel[np.argsort(dloc[sel], kind="stable")]
        gs, dl, w = gsrc[order], dloc[order], ew[order]
        blk = dl // 128
        rel = dl % 128
        blocks = []
        for b in range(n_blk):
            m = blk == b
            in_a = gs[m] < half
            a = (gs[m][in_a], rel[m][in_a], w[m][in_a])
            bb = (gs[m][~in_a] - half, rel[m][~in_a], w[m][~in_a])
            blocks.append((a, bb))
            ca_max = max(ca_max, -(-len(a[0]) // 128))
            cb_max = max(cb_max, -(-len(bb[0]) // 128))
        per_core.append(blocks)

    CA, CB = ca_max, cb_max
    C = CA + CB
    planes = []
    for k in range(NC):
        idx_a = np.zeros(n_blk * CA * 128, np.int16)
        idx_b = np.zeros(n_blk * CB * 128, np.int16)
        # host-built one-hot W_sel plane: wsel[chunk, e, d] = ew * (dst == d)
        # (pad edges keep ew=0).  Streamed from DRAM on device instead of
        # being rebuilt with DVE is_eq/mult mega-ops every layer.
        dsti = np.zeros((n_blk * C, 128), np.int64)
        ewf = np.zeros((n_blk * C, 128), np.float32)
        for b, (a, bb) in enumerate(per_core[k]):
            na, nb = len(a[0]), len(bb[0])
            idx_a[b * CA * 128:b * CA * 128 + na] = a[0].astype(np.int16)
            idx_b[b * CB * 128:b * CB * 128 + nb] = bb[0].astype(np.int16)
            for (cnt, off, rels, ws) in ((na, 0, a[1], a[2]),
                                         (nb, CA, bb[1], bb[2])):
                if cnt == 0:
                    continue
                e = np.arange(cnt)
                chunk = b * C + off + e // 128
                dsti[chunk, e % 128] = rels.astype(np.int64)
                ewf[chunk, e % 128] = ws
        tot = n_blk * C
        ew16 = ewf.astype(ml_dtypes.bfloat16)
        W = np.zeros((tot, 128, 128), ml_dtypes.bfloat16)
        ch = np.arange(tot)[:, None]
        ee = np.arange(128)[None, :]
        W[ch, ee, dsti] = ew16
        wsel16 = np.ascontiguousarray(
            W.transpose(1, 0, 2).reshape(128, tot * 128))
        planes.append(dict(
            idx_a=_wrap_idx(idx_a),
            idx_b=_wrap_idx(idx_b),
            wsel16=wsel16,
            wsel8=wsel16.astype(ml_dtypes.float8_e4m3),
        ))
    return planes, CA, CB


def _pad_nodes(arr, cfg):
    """[N, F] -> [NC*SHARD_PAD, F] with zero-filled pad rows per shard."""
    f = arr.shape[1]
    out = np.zeros((NC * cfg["SHARD_PAD"], f), arr.dtype)
    for k in range(NC):
        out[k * cfg["SHARD_PAD"]:k * cfg["SHARD_PAD"] + cfg["SHARD"]] = (
            arr[k * cfg["SHARD"]:(k + 1) * cfg["SHARD"]]
        )
    return out


# --------------------------------------------------------------------------
# Device program
# --------------------------------------------------------------------------

def build_program(cfg, CA, CB, CDA, CDB):
    n_blk = cfg["SHARD_PAD"] // 128
    npad = NC * cfg["SHARD_PAD"]
    half = npad // 2
    B = cfg["B"]
    C = CA + CB
    CD = CDA + CDB

    nc = bacc.Bacc("TRN2", target_bir_lowering=False, debug=False,
                   num_devices=NC, num_swdge_queues=4)

    def din(name, shape, dt=F32):
        return nc.declare_dram_parameter(name, list(shape), dt, isOutput=False)

    x_pad = din("x_pad", [npad, 128], BF16)
    x_localT = din("x_localT", [128, cfg["SHARD_PAD"]])
    dur_pad = din("dur_pad", [npad, 128], BF16)
    dur_localT = din("dur_localT", [64, cfg["SHARD_PAD"]])
    ev_idx_a = din("ev_idx_a", [128, n_blk * CA * 8], I16)
    ev_idx_b = din("ev_idx_b", [128, n_blk * CB * 8], I16)
    ev_wsel16 = din("ev_wsel16", [128, n_blk * C * 128], BF16)
    ev_wsel8 = din("ev_wsel8", [128, n_blk * C * 128], F8)
    du_idx_a = din("du_idx_a", [128, n_blk * CDA * 8], I16)
    du_idx_b = din("du_idx_b", [128, n_blk * CDB * 8], I16)
    du_wsel16 = din("du_wsel16", [128, n_blk * CD * 128], BF16)
    ssel_in = din("ssel", [128, n_blk * B])
    seq_in = din("seq_features", [B, 256])

    wnames = [
        ("g1_Wr", [128, 128]), ("g1_br", [128, 1]), ("g1_Wroot", [128, 128]),
        ("g2_Wr", [128, 128]), ("g2_br", [128, 1]), ("g2_Wroot", [128, 128]),
        ("d1_Wr", [64, 128]), ("d1_br", [128, 1]), ("d1_Wroot", [64, 128]),
        ("c1_Wr", [256, 256]), ("c1_Wroot", [256, 256]),
        ("skip_W", [256, 256]), ("bias_c_row", [1, 256]),
        ("fc1_W", [256, 256]), ("fc1_b", [256, 1]),
        ("fc2_W", [256, 128]), ("fc2_b", [128, 1]),
        ("fcc_W", [384, 256]), ("fcc_b", [256, 1]),
        ("cls_W", [256, 16]), ("cls_b_rep", [B, 16]),
    ]
    wdram = {nm: din(nm, sh) for nm, sh in wnames}
    out_ext = nc.declare_dram_parameter("out", [B, 16], F32, isOutput=True)

    from contextlib import ExitStack
    with tile.TileContext(nc) as tc, ExitStack() as ctx:
        cpool = ctx.enter_context(tc.tile_pool(name="const", bufs=1))
        spool = ctx.enter_context(tc.tile_pool(name="sbuf", bufs=3))
        wpool = ctx.enter_context(tc.tile_pool(name="wsel", bufs=2))
        gpool = ctx.enter_context(tc.tile_pool(name="gath", bufs=3))
        ppool = ctx.enter_context(tc.tile_pool(name="psum", bufs=2,
                                               space="PSUM"))
        ppool2 = ctx.enter_context(tc.tile_pool(name="psum2", bufs=2,
                                                space="PSUM"))
        pagg = ctx.enter_context(tc.tile_pool(name="pagg", bufs=2,
                                              space="PSUM"))
        pacc = ctx.enter_context(tc.tile_pool(name="pacc", bufs=1,
                                              space="PSUM"))
        dpool = ctx.enter_context(tc.tile_pool(name="dram", bufs=1,
                                               space="DRAM"))

        # ---- constants -----------------------------------------------------
        ident = cpool.tile([128, 128], F32, tag="ident")
        make_identity(nc, ident[:])
        ones_row = cpool.tile([1, 128], F32, tag="ones_row")
        nc.vector.memset(ones_row[:], 1.0)

        def wtiles(nm, rows, cols):
            ts = []
            for i in range(0, rows, 128):
                p = min(128, rows - i)
                t = cpool.tile([p, cols], F32, tag=f"w_{nm}_{i}")
                nc.sync.dma_start(out=t[:], in_=wdram[nm][i:i + p, :])
                ts.append(t)
            return ts

        ev_ia = cpool.tile([128, n_blk * CA * 8], I16, tag="ev_ia")
        nc.sync.dma_start(out=ev_ia[:], in_=ev_idx_a[:])
        ev_ib = cpool.tile([128, n_blk * CB * 8], I16, tag="ev_ib")
        nc.sync.dma_start(out=ev_ib[:], in_=ev_idx_b[:])
        du_ia = cpool.tile([128, n_blk * CDA * 8], I16, tag="du_ia")
        nc.sync.dma_start(out=du_ia[:], in_=du_idx_a[:])
        du_ib = cpool.tile([128, n_blk * CDB * 8], I16, tag="du_ib")
        nc.sync.dma_start(out=du_ib[:], in_=du_idx_b[:])
        ssel = cpool.tile([128, n_blk * B], F32, tag="ssel")
        nc.sync.dma_start(out=ssel[:], in_=ssel_in[:])

        W = {}
        for nm, sh in wnames:
            W[nm] = wtiles(nm, sh[0], sh[1])

        def load_wsel(wsel, wsel_dram, c0, nchunk):
            """Stream nchunk host-built one-hot chunks from DRAM."""
            nc.sync.dma_start(
                out=wsel[:, :nchunk * 128],
                in_=wsel_dram[:, c0 * 128:(c0 + nchunk) * 128])

        # fold c1_Wroot + skip_W (both multiply xcT in c1 stage2)
        W_rs = []
        for kh in range(2):
            t = cpool.tile([128, 256], F32, tag=f"w_rs_{kh}")
            nc.vector.tensor_add(out=t[:], in0=W["c1_Wroot"][kh][:],
                                 in1=W["skip_W"][kh][:])
            W_rs.append(t)

        # ---- DRAM intermediates -------------------------------------------
        sp = cfg["SHARD_PAD"]
        h1T_local = dpool.tile([128, sp], F32, tag="h1T_local")
        h1b_local = dpool.tile([sp, 128], BF16, tag="h1b_local")
        h1_full = dpool.tile([npad, 128], BF16, tag="h1_full",
                             addr_space="Shared")
        # combined [h2|d] bf16 table (g2 writes cols 0:128, d1 cols 128:256)
        hd_local = dpool.tile([sp, 256], F8, tag="hd_local")
        hd_full = dpool.tile([npad, 256], F8, tag="hd_full",
                             addr_space="Shared")
        h2T_local = dpool.tile([128, sp], F32, tag="h2T_local")
        dT_local = dpool.tile([128, sp], F32, tag="dT_local")
        ar_in = dpool.tile([B, 257], F32, tag="ar_in")
        ar_out = dpool.tile([B, 257], F32, tag="ar_out", addr_space="Shared")

        # ---- generic GraphConv layer (F_out = 128) ------------------------
        def g_layer(lname, fin, gfin, tbl, idx_a, idx_b, ca, cb, wsel_dram,
                    prevT_dram, wr, wroot, br, outT_dram, out_bf, out_bf_col,
                    npair=4):
            c = ca + cb
            for bp in range(0, n_blk, npair):
                np_ = min(npair, n_blk - bp)
                tot = np_ * c
                gath = gpool.tile([128, tot * gfin], BF16, tag="gath")
                for (qn, (idx, cc, off, th)) in enumerate((
                    (idx_a, ca, 0, tbl[0]),
                    (idx_b, cb, np_ * ca, tbl[1]),
                )):
                    span = np_ * cc
                    lo = span // 2
                    for (sq, c0, c1) in ((qn, 0, lo), (qn + 2, lo, span)):
                        if c1 == c0:
                            continue
                        nc.gpsimd.dma_gather(
                            out_ap=gath[:, (off + c0) * gfin:
                                        (off + c1) * gfin].rearrange(
                                "p (c f) -> p c f", c=c1 - c0),
                            in_ap=th,
                            idxs_ap=idx[:, bp * cc * 8 + c0 * 8:
                                        bp * cc * 8 + c1 * 8],
                            num_idxs=(c1 - c0) * 128,
                            num_idxs_reg=(c1 - c0) * 128,
                            elem_size=gfin,
                            single_packet=False,
                            queue_num=sq,
                        )
                gmm = gath

                wsel = wpool.tile([128, np_ * c * 128], BF16, tag="wsel")
                load_wsel(wsel, wsel_dram, bp * c, np_ * c)

                for r in range(np_):
                    b = bp + r

                    def gpos(j, r=r):
                        if j < ca:
                            return r * ca + j
                        return np_ * ca + r * cb + (j - ca)

                    agg_ps = pagg.tile([fin, 128], F32, tag="agg_ps")
                    for j in range(c):
                        g0 = gpos(j) * gfin
                        nc.tensor.matmul(
                            out=agg_ps[:],
                            lhsT=gmm[:, g0:g0 + fin],
                            rhs=wsel[:, (r * c + j) * 128:
                                     (r * c + j + 1) * 128],
                            start=(j == 0), stop=(j == c - 1))
                    agg = spool.tile([fin, 128], F32, tag="agg_sb")
                    nc.scalar.copy(agg[:], agg_ps[:])

                    prevT = spool.tile([fin, 128], F32, tag="prevT")
                    nc.sync.dma_start(
                        out=prevT[:],
                        in_=prevT_dram[:, b * 128:(b + 1) * 128])

                    o_ps = ppool2.tile([128, 128], F32, tag="o")
                    nc.tensor.matmul(out=o_ps[:], lhsT=wr[0][:], rhs=agg[:],
                                     start=True, stop=False)
                    nc.tensor.matmul(out=o_ps[:], lhsT=wroot[0][:],
                                     rhs=prevT[:], start=False, stop=True)
                    oT = spool.tile([128, 128], F32, tag="oT_sb")
                    nc.scalar.activation(out=oT[:], in_=o_ps[:], func=AF.Relu,
                                         bias=br[0][:, :1], scale=1.0)
                    nc.sync.dma_start(
                        out=outT_dram[:, b * 128:(b + 1) * 128], in_=oT[:])
                    # node-major bf16 copy for the next gather table
                    nm_ps = ppool.tile([128, 128], F32, tag="tp")
                    nc.tensor.transpose(out=nm_ps[:], in_=oT[:],
                                        identity=ident[:])
                    obf = spool.tile([128, 128], out_bf.dtype,
                                     tag="obf_sb")
                    nc.scalar.copy(obf[:], nm_ps[:])
                    nc.sync.dma_start(
                        out=out_bf[b * 128:(b + 1) * 128,
                                   out_bf_col:out_bf_col + 128],
                        in_=obf[:])

        # ---- g1 ------------------------------------------------------------
        g_layer("g1", 128, 128, (x_pad[0:half, :], x_pad[half:npad, :]),
                ev_ia, ev_ib, CA, CB, ev_wsel16,
                x_localT, W["g1_Wr"], W["g1_Wroot"], W["g1_br"],
                h1T_local, h1b_local, 0)
        nc.gpsimd.collective_compute(
            "AllGather", OP.bypass, replica_groups=[list(range(NC))],
            ins=[h1b_local.opt()], outs=[h1_full.opt()])

        # ---- d1 (independent of h1; overlaps the AllGather) ---------------
        g_layer("d1", 64, 128, (dur_pad[0:half, :], dur_pad[half:npad, :]),
                du_ia, du_ib, CDA, CDB, du_wsel16,
                dur_localT, W["d1_Wr"], W["d1_Wroot"], W["d1_br"],
                dT_local, hd_local, 128)

        # ---- g2 ------------------------------------------------------------
        g_layer("g2", 128, 128, (h1_full[0:half, :], h1_full[half:npad, :]),
                ev_ia, ev_ib, CA, CB, ev_wsel16,
                h1T_local, W["g2_Wr"], W["g2_Wroot"], W["g2_br"],
                h2T_local, hd_local, 0)

        nc.gpsimd.collective_compute(
            "AllGather", OP.bypass, replica_groups=[list(range(NC))],
            ins=[hd_local.opt()], outs=[hd_full.opt()])

        # ---- c1 + pooling --------------------------------------------------
        pooled = cpool.tile([B, 257], F32, tag="pooled")
        nc.vector.memset(pooled[:], 0.0)
        tbl = (hd_full[0:half, :], hd_full[half:npad, :])
        for bp in range(0, n_blk, 2):
            np_ = min(2, n_blk - bp)
            ctot = np_ * C
            gath = gpool.tile([128, ctot * 256], F8, tag="gath")
            for (qn, (idx, cc, off, th)) in enumerate((
                (ev_ia, CA, 0, tbl[0]),
                (ev_ib, CB, np_ * CA, tbl[1]),
            )):
                span = np_ * cc
                lo = span // 2
                for (sq, c0, c1) in ((qn, 0, lo), (qn + 2, lo, span)):
                    if c1 == c0:
                        continue
                    nc.gpsimd.dma_gather(
                        out_ap=gath[:, (off + c0) * 256:
                                    (off + c1) * 256].rearrange(
                            "p (c f) -> p c f", c=c1 - c0),
                        in_ap=th,
                        idxs_ap=idx[:, bp * cc * 8 + c0 * 8:
                                    bp * cc * 8 + c1 * 8],
                        num_idxs=(c1 - c0) * 128,
                        num_idxs_reg=(c1 - c0) * 128,
                        elem_size=256,
                        single_packet=False,
                        queue_num=sq,
                    )
            wsel = wpool.tile([128, np_ * C * 128], F8, tag="wsel")
            nc.sync.dma_start(
                out=wsel[:, :np_ * C * 128],
                in_=ev_wsel8[:, bp * C * 128:(bp * C + np_ * C) * 128])

            for r in range(np_):
                b = bp + r

                def gpos_c1(j, r=r, np_=np_):
                    if j < CA:
                        return r * CA + j
                    return np_ * CA + r * CB + (j - CA)

                agg_f_ps = pacc.tile([128, 128], F32, tag="agg_f")
                agg_d_ps = pacc.tile([128, 128], F32, tag="agg_d")
                for j in range(C):
                    g0 = gpos_c1(j) * 256
                    ws = wsel[:, (r * C + j) * 128:(r * C + j + 1) * 128]
                    nc.tensor.matmul(
                        out=agg_f_ps[:], lhsT=gath[:, g0:g0 + 128],
                        rhs=ws, start=(j == 0), stop=(j == C - 1))
                    nc.tensor.matmul(
                        out=agg_d_ps[:], lhsT=gath[:, g0 + 128:g0 + 256],
                        rhs=ws, start=(j == 0), stop=(j == C - 1))
                agg_f = spool.tile([128, 128], F32, tag="aggc_f")
                nc.scalar.copy(agg_f[:], agg_f_ps[:])
                agg_d = spool.tile([128, 128], F32, tag="aggc_d")
                nc.scalar.copy(agg_d[:], agg_d_ps[:])
                aggs = (agg_f, agg_d)

                xcT = []
                for kh, src_dram in ((0, h2T_local), (1, dT_local)):
                    t_sb = spool.tile([128, 128], F32, tag=f"xcT_sb{kh}")
                    nc.sync.dma_start(
                        out=t_sb[:],
                        in_=src_dram[:, b * 128:(b + 1) * 128])
                    xcT.append(t_sb)

                # node-major y: out[dst, fout] accumulated in one PSUM tile;
                # bias added via a rank-1 ones-row matmul, so relu needs no
                # per-partition bias
                y_ps = ppool2.tile([128, 256], F32, tag="o")
                nc.tensor.matmul(out=y_ps[:], lhsT=ones_row[:1, :],
                                 rhs=W["bias_c_row"][0][:1, :],
                                 start=True, stop=False)
                for kh in range(2):
                    nc.tensor.matmul(out=y_ps[:], lhsT=aggs[kh][:],
                                     rhs=W["c1_Wr"][kh][:], start=False,
                                     stop=False)
                    nc.tensor.matmul(out=y_ps[:], lhsT=xcT[kh][:],
                                     rhs=W_rs[kh][:], start=False,
                                     stop=(kh == 1))
                y_nm = spool.tile([128, 272], F32, tag="y_nm")
                nc.vector.memset(y_nm[:, 256:257], 1.0)
                nc.scalar.activation(out=y_nm[:, :256], in_=y_ps[:],
                                     func=AF.Relu)
                pool_ps = ppool2.tile([B, 257], F32, tag="o")
                nc.tensor.matmul(out=pool_ps[:],
                                 lhsT=ssel[:, b * B:(b + 1) * B],
                                 rhs=y_nm[:, :257], start=True, stop=True)
                nc.vector.tensor_add(out=pooled[:], in0=pooled[:],
                                     in1=pool_ps[:])

        # ---- AllReduce pooled sums + counts -------------------------------
        nc.sync.dma_start(out=ar_in[:], in_=pooled[:])
        nc.gpsimd.collective_compute(
            "AllReduce", OP.add, replica_groups=[list(range(NC))],
            ins=[ar_in.opt()], outs=[ar_out.opt()])

        # ---- head (replicated on every core) ------------------------------
        pl = spool.tile([B, 257], F32, tag="pl")
        nc.sync.dma_start(out=pl[:], in_=ar_out[:])
        cnt = spool.tile([B, 1], F32, tag="cnt")
        nc.vector.tensor_scalar(out=cnt[:], in0=pl[:, 256:257], scalar1=1.0,
                                scalar2=None, op0=OP.max)
        rec = spool.tile([B, 1], F32, tag="rec")
        nc.vector.reciprocal(out=rec[:], in_=cnt[:])
        emb = spool.tile([B, 256], F32, tag="emb")
        nc.vector.tensor_tensor(out=emb[:], in0=pl[:, :256],
                                in1=rec[:, :1].to_broadcast([B, 256]),
                                op=OP.mult)

        def transpose_2(src, tag):
            ts = []
            for kh in range(2):
                t_ps = ppool.tile([128, B], F32, tag="tp")
                nc.tensor.transpose(out=t_ps[:],
                                    in_=src[:, kh * 128:(kh + 1) * 128],
                                    identity=ident[:B, :B])
                t_sb = spool.tile([128, B], F32, tag=f"{tag}_sb{kh}")
                nc.vector.tensor_copy(t_sb[:], t_ps[:])
                ts.append(t_sb)
            return ts

        embT = transpose_2(emb, "embT")

        seq = spool.tile([B, 256], F32, tag="seq")
        nc.sync.dma_start(out=seq[:], in_=seq_in[:])
        seqT = transpose_2(seq, "seqT")

        def mlp(rhss, wname, bname, act, out_halves, tag):
            outs = []
            for o in range(out_halves):
                osl = slice(o * 128, (o + 1) * 128)
                ps = ppool2.tile([128, B], F32, tag="o")
                for si, r in enumerate(rhss):
                    nc.tensor.matmul(out=ps[:], lhsT=W[wname][si][:, osl],
                                     rhs=r[:], start=(si == 0),
                                     stop=(si == len(rhss) - 1))
                t = spool.tile([128, B], F32, tag=f"{tag}_sb{o}")
                nc.scalar.activation(out=t[:], in_=ps[:], func=act,
                                     bias=W[bname][o][:, :1], scale=1.0)
                outs.append(t)
            return outs

        s1T = mlp(seqT, "fc1_W", "fc1_b", AF.Relu, 2, "s1")
        sT = mlp(s1T, "fc2_W", "fc2_b", AF.Relu, 1, "s2")
        hT = mlp(embT + sT, "fcc_W", "fcc_b", AF.Relu, 2, "hc")

        lg_ps = ppool2.tile([B, 16], F32, tag="o")
        for o in range(2):
            nc.tensor.matmul(out=lg_ps[:], lhsT=hT[o][:], rhs=W["cls_W"][o][:],
                             start=(o == 0), stop=(o == 1))
        logits = spool.tile([B, 16], F32, tag="logits")
        nc.vector.tensor_tensor(out=logits[:], in0=lg_ps[:],
                                in1=W["cls_b_rep"][0][:], op=OP.add)
        rmax = spool.tile([B, 1], F32, tag="rmax")
        nc.vector.tensor_reduce(out=rmax[:], in_=logits[:],
                                axis=mybir.AxisListType.X, op=OP.max)
        tshift = spool.tile([B, 16], F32, tag="tshift")
        nc.vector.tensor_scalar(out=tshift[:], in0=logits[:],
                                scalar1=rmax[:, :1], scalar2=None,
                                op0=OP.subtract)
        ex = spool.tile([B, 16], F32, tag="ex")
        nc.scalar.activation(out=ex[:], in_=tshift[:], func=AF.Exp)
        esum = spool.tile([B, 1], F32, tag="esum")
        nc.vector.tensor_reduce(out=esum[:], in_=ex[:],
                                axis=mybir.AxisListType.X, op=OP.add)
        lsum = spool.tile([B, 1], F32, tag="lsum")
        nc.scalar.activation(out=lsum[:], in_=esum[:], func=AF.Ln)
        res = spool.tile([B, 16], F32, tag="res")
        nc.vector.tensor_scalar(out=res[:], in0=tshift[:],
                                scalar1=lsum[:, :1], scalar2=None,
                                op0=OP.subtract)
        nc.sync.dma_start(out=out_ext[:], in_=res[:])

    nc.compile()
    return nc


# --------------------------------------------------------------------------
# Host orchestration
# --------------------------------------------------------------------------

def make_in_maps(inputs, cfg):
    import ml_dtypes
    x = np.asarray(inputs["x"], np.float32)
    # Reference masks x at -1.0 sentinels (and the post-layer masks are
    # no-ops given relu(-1.0) == 0), so pre-mask on host once.
    x = np.where(x == -1.0, 0.0, x)
    dur_x = np.asarray(inputs["dur_x"], np.float32)
    batch = np.asarray(inputs["batch"], np.int64)

    ev_planes, CA, CB = prep_edges(inputs["edge_index"], inputs["edge_attr"],
                                   cfg)
    du_planes, CDA, CDB = prep_edges(inputs["dur_edge_index"],
                                     inputs["dur_edge_attr"], cfg)

    x_pad_f32 = _pad_nodes(x, cfg)
    x_pad = x_pad_f32.astype(ml_dtypes.bfloat16)
    dur_padded = _pad_nodes(dur_x, cfg)
    # bf16 dur table padded to 128 features (256B rows for the gather)
    dur_pad_bf16 = np.zeros((dur_padded.shape[0], 128), ml_dtypes.bfloat16)
    dur_pad_bf16[:, :64] = dur_padded.astype(ml_dtypes.bfloat16)

    n_blk = cfg["SHARD_PAD"] // 128
    B = cfg["B"]
    bias_c = (np.asarray(inputs["c1_br"], np.float32)
              + np.asarray(inputs["skip_b"], np.float32))

    def col(v):
        return np.ascontiguousarray(
            np.asarray(v, np.float32).reshape(-1, 1))

    weights = dict(
        g1_Wr=inputs["g1_Wr"], g1_br=col(inputs["g1_br"]),
        g1_Wroot=inputs["g1_Wroot"],
        g2_Wr=inputs["g2_Wr"], g2_br=col(inputs["g2_br"]),
        g2_Wroot=inputs["g2_Wroot"],
        d1_Wr=inputs["d1_Wr"], d1_br=col(inputs["d1_br"]),
        d1_Wroot=inputs["d1_Wroot"],
        c1_Wr=inputs["c1_Wr"], c1_Wroot=inputs["c1_Wroot"],
        skip_W=inputs["skip_W"],
        bias_c_row=np.asarray(bias_c, np.float32).reshape(1, -1),
        fc1_W=inputs["fc1_W"], fc1_b=col(inputs["fc1_b"]),
        fc2_W=inputs["fc2_W"], fc2_b=col(inputs["fc2_b"]),
        fcc_W=inputs["fcc_W"], fcc_b=col(inputs["fcc_b"]),
        cls_W=inputs["cls_W"],
        cls_b_rep=np.tile(np.asarray(inputs["cls_b"], np.float32)[None, :],
                          (B, 1)),
        seq_features=inputs["seq_features"],
    )
    weights = {k: np.ascontiguousarray(np.asarray(v, np.float32))
               for k, v in weights.items()}

    in_maps = []
    for k in range(NC):
        sp = cfg["SHARD_PAD"]
        # graph-membership one-hot [128 node-in-block, n_blk * B]
        bfr_flat = np.full(sp, -1, np.int64)
        bfr_flat[:cfg["SHARD"]] = batch[k * cfg["SHARD"]:(k + 1) * cfg["SHARD"]]
        ssel = np.zeros((n_blk, 128, B), np.float32)
        bb = bfr_flat.reshape(n_blk, 128)
        blk_i, pos_i = np.nonzero(bb >= 0)
        ssel[blk_i, pos_i, bb[blk_i, pos_i]] = 1.0
        ssel = np.ascontiguousarray(
            ssel.transpose(1, 0, 2).reshape(128, n_blk * B))

        m = dict(
            x_pad=x_pad,
            x_localT=np.ascontiguousarray(
                x_pad_f32[k * sp:(k + 1) * sp].T),
            dur_pad=dur_pad_bf16,
            dur_localT=np.ascontiguousarray(
                dur_padded[k * sp:(k + 1) * sp].T),
            ev_idx_a=ev_planes[k]["idx_a"], ev_idx_b=ev_planes[k]["idx_b"],
            ev_wsel16=ev_planes[k]["wsel16"], ev_wsel8=ev_planes[k]["wsel8"],
            du_idx_a=du_planes[k]["idx_a"], du_idx_b=du_planes[k]["idx_b"],
            du_wsel16=du_planes[k]["wsel16"],
            ssel=ssel,
            **weights,
        )
        in_maps.append(m)
    return in_maps, (CA, CB, CDA, CDB)


_LAST_RESULT = None


def kernel(**inputs) -> np.ndarray:
    global _LAST_RESULT
    cfg = dict(REAL)
    cfg["N"] = inputs["x"].shape[0]
    cfg["B"] = inputs["seq_features"].shape[0]
    in_maps, (CA, CB, CDA, CDB) = make_in_maps(inputs, cfg)
    nc = build_program(cfg, CA, CB, CDA, CDB)
    from concourse.bass_utils import run_bass_kernel_spmd
    res = run_bass_kernel_spmd(nc, in_maps, list(range(NC)))
    _LAST_RESULT = res
    return np.asarray(res.results[0]["out"], np.float32)



# revision 31
# speedup vs baseline: 1.0231x; 1.0231x over previous
"""Trainium2 Bass kernel for nn_EventSequenceDurationGraphConvModel.

Self-contained: accepts FULL inputs, shards across 8 NeuronCores internally
(nodes/edges partitioned by destination node per core), runs one SPMD Bass
program, and returns the FULL [64, 16] output.

Per-core GraphConv layers aggregate via dma_gather of source rows (bf16)
followed by PSUM matmuls against one-hot selection matrices
W_sel[e, d] = ew[e] * (dst_rel[e] == d), so the segment_sum needs no
scatter. Key performance structure:
  - SWDGE descriptor dispatch is the machine bottleneck (~5ns/descriptor,
    one descriptor per gathered edge row). Gathers are spread across 4
    SWDGE queues (ucode max), which roughly halves effective dispatch
    time vs a single queue. single_packet must stay False (True crashes).
  - W_sel matrices are built on-device with TWO broadcast tensor_tensor
    mega-ops per block group (is_equal against an iota row, then in-place
    multiply by edge weights) -- cheap on DVE and no HBM traffic to
    contend with gather descriptor dispatch.
  - c1 gathers ONE combined [h2|d] table with one descriptor per edge
    instead of two, saving ~20% of all descriptors. The table is FP8
    (256B rows): halves the g2->c1 AllGather volume (the one serial
    collective bubble) and c1's gather bytes; aggregation noise is
    diluted by exact f32 root terms and the 780-node mean pool
    (measured end-to-end rel err 6.6e-4 vs 2e-2 budget).
  - dur table is bf16 padded to 128 features so d1's gather needs no
    f32 download + cast.
  - Root-term inputs stay feature-major ([F, nodes]); c1's output stage
    computes node-major y directly (swapped matmul operands, bias folded
    in via a rank-1 ones-row matmul), so no PE transposes there.
  - Host pre-masks x (x == -1.0 -> 0); the reference's post-layer mask
    ops are no-ops given relu(-1.0) == 0, so no device masking at all.
  - gather tiles are triple-buffered so descriptor generation for group
    n+2 overlaps compute of group n.

Pipeline per core (fp32 accumulation, bf16 gathers/matmuls):
  g1 -> AllGather(h1) overlapped with d1 -> g2 -> AllGather([h2|d]) -> c1
  -> pool (PSUM matmul against host-built one-hot graph membership)
  -> AllReduce -> replicated MLP head + log_softmax.
"""
import sys

import numpy as np

sys.path.insert(0, "/opt/trn_rl_repo")

from concourse import bacc, bass, mybir  # noqa: E402
import concourse.tile as tile  # noqa: E402
from concourse.masks import make_identity  # noqa: E402

F32 = mybir.dt.float32
BF16 = mybir.dt.bfloat16
F8 = mybir.dt.float8e4
I16 = mybir.dt.int16
AF = mybir.ActivationFunctionType
OP = mybir.AluOpType

NC = 8

REAL = dict(N=50000, E=800000, B=64, SHARD=6250, SHARD_PAD=6272)


# --------------------------------------------------------------------------
# Host-side sharding / preprocessing (pure index/layout work)
# --------------------------------------------------------------------------

def _gpid(node_id, cfg):
    """Real node id -> padded global id."""
    return (node_id // cfg["SHARD"]) * cfg["SHARD_PAD"] + node_id % cfg["SHARD"]


def _wrap_idx(flat_i16):
    """Flat int16 index list -> dma_gather plane [128, n/16] (16-part wrap,
    replicated across the 8 gpsimd cores)."""
    n = flat_i16.shape[0]
    assert n % 16 == 0
    return np.tile(flat_i16.reshape(n // 16, 16).T, (8, 1)).copy()


def prep_edges(edge_index, edge_attr, cfg):
    """Shard + sort + pad the edge list. Returns per-core gather planes,
    host-built W_sel planes, and uniform per-block chunk counts (CA, CB)."""
    import ml_dtypes
    n_blk = cfg["SHARD_PAD"] // 128
    half = NC * cfg["SHARD_PAD"] // 2
    src = np.asarray(edge_index[0], dtype=np.int64)
    dst = np.asarray(edge_index[1], dtype=np.int64)
    ew = np.asarray(edge_attr, dtype=np.float32)
    gsrc = _gpid(src, cfg)
    core = dst // cfg["SHARD"]
    dloc = dst % cfg["SHARD"]

    per_core = []
    ca_max = cb_max = 1
    for k in range(NC):
        sel = np.nonzero(core == k)[0]
        order = sel[np.argsort(dloc[sel], kind="stable")]
        gs, dl, w = gsrc[order], dloc[order], ew[order]
        blk = dl // 128
        rel = dl % 128
        blocks = []
        for b in range(n_blk):
            m = blk == b
            in_a = gs[m] < half
            a = (gs[m][in_a], rel[m][in_a], w[m][in_a])
            bb = (gs[m][~in_a] - half, rel[m][~in_a], w[m][~in_a])
            blocks.append((a, bb))
            ca_max = max(ca_max, -(-len(a[0]) // 128))
            cb_max = max(cb_max, -(-len(bb[0]) // 128))
        per_core.append(blocks)

    CA, CB = ca_max, cb_max
    C = CA + CB
    planes = []
    for k in range(NC):
        idx_a = np.zeros(n_blk * CA * 128, np.int16)
        idx_b = np.zeros(n_blk * CB * 128, np.int16)
        # host-built one-hot W_sel plane: wsel[chunk, e, d] = ew * (dst == d)
        # (pad edges keep ew=0).  Streamed from DRAM on device instead of
        # being rebuilt with DVE is_eq/mult mega-ops every layer.
        dsti = np.zeros((n_blk * C, 128), np.int64)
        ewf = np.zeros((n_blk * C, 128), np.float32)
        for b, (a, bb) in enumerate(per_core[k]):
            na, nb = len(a[0]), len(bb[0])
            idx_a[b * CA * 128:b * CA * 128 + na] = a[0].astype(np.int16)
            idx_b[b * CB * 128:b * CB * 128 + nb] = bb[0].astype(np.int16)
            for (cnt, off, rels, ws) in ((na, 0, a[1], a[2]),
                                         (nb, CA, bb[1], bb[2])):
                if cnt == 0:
                    continue
                e = np.arange(cnt)
                chunk = b * C + off + e // 128
                dsti[chunk, e % 128] = rels.astype(np.int64)
                ewf[chunk, e % 128] = ws
        tot = n_blk * C
        ew16 = ewf.astype(ml_dtypes.bfloat16)
        W = np.zeros((tot, 128, 128), ml_dtypes.bfloat16)
        ch = np.arange(tot)[:, None]
        ee = np.arange(128)[None, :]
        W[ch, ee, dsti] = ew16
        wsel16 = np.ascontiguousarray(
            W.transpose(1, 0, 2).reshape(128, tot * 128))
        planes.append(dict(
            idx_a=_wrap_idx(idx_a),
            idx_b=_wrap_idx(idx_b),
            wsel16=wsel16,
            wsel8=wsel16.astype(ml_dtypes.float8_e4m3),
        ))
    return planes, CA, CB


def _pad_nodes(arr, cfg):
    """[N, F] -> [NC*SHARD_PAD, F] with zero-filled pad rows per shard."""
    f = arr.shape[1]
    out = np.zeros((NC * cfg["SHARD_PAD"], f), arr.dtype)
    for k in range(NC):
        out[k * cfg["SHARD_PAD"]:k * cfg["SHARD_PAD"] + cfg["SHARD"]] = (
            arr[k * cfg["SHARD"]:(k + 1) * cfg["SHARD"]]
        )
    return out


# --------------------------------------------------------------------------
# Device program
# --------------------------------------------------------------------------

def build_program(cfg, CA, CB, CDA, CDB):
    n_blk = cfg["SHARD_PAD"] // 128
    npad = NC * cfg["SHARD_PAD"]
    half = npad // 2
    B = cfg["B"]
    C = CA + CB
    CD = CDA + CDB

    nc = bacc.Bacc("TRN2", target_bir_lowering=False, debug=False,
                   num_devices=NC, num_swdge_queues=4)

    def din(name, shape, dt=F32):
        return nc.declare_dram_parameter(name, list(shape), dt, isOutput=False)

    x_pad = din("x_pad", [npad, 128], BF16)
    x_localT = din("x_localT", [128, cfg["SHARD_PAD"]])
    dur_pad = din("dur_pad", [npad, 128], BF16)
    dur_localT = din("dur_localT", [64, cfg["SHARD_PAD"]])
    ev_idx_a = din("ev_idx_a", [128, n_blk * CA * 8], I16)
    ev_idx_b = din("ev_idx_b", [128, n_blk * CB * 8], I16)
    ev_wsel16 = din("ev_wsel16", [128, n_blk * C * 128], BF16)
    ev_wsel8 = din("ev_wsel8", [128, n_blk * C * 128], F8)
    du_idx_a = din("du_idx_a", [128, n_blk * CDA * 8], I16)
    du_idx_b = din("du_idx_b", [128, n_blk * CDB * 8], I16)
    du_wsel16 = din("du_wsel16", [128, n_blk * CD * 128], BF16)
    ssel_in = din("ssel", [128, n_blk * B])
    seq_in = din("seq_features", [B, 256])

    wnames = [
        ("g1_Wr", [128, 128]), ("g1_br", [128, 1]), ("g1_Wroot", [128, 128]),
        ("g2_Wr", [128, 128]), ("g2_br", [128, 1]), ("g2_Wroot", [128, 128]),
        ("d1_Wr", [64, 128]), ("d1_br", [128, 1]), ("d1_Wroot", [64, 128]),
        ("c1_Wr", [256, 256]), ("c1_Wroot", [256, 256]),
        ("skip_W", [256, 256]), ("bias_c_row", [1, 256]),
        ("fc1_W", [256, 256]), ("fc1_b", [256, 1]),
        ("fc2_W", [256, 128]), ("fc2_b", [128, 1]),
        ("fcc_W", [384, 256]), ("fcc_b", [256, 1]),
        ("cls_W", [256, 16]), ("cls_b_rep", [B, 16]),
    ]
    wdram = {nm: din(nm, sh) for nm, sh in wnames}
    out_ext = nc.declare_dram_parameter("out", [B, 16], F32, isOutput=True)

    from contextlib import ExitStack
    with tile.TileContext(nc) as tc, ExitStack() as ctx:
        cpool = ctx.enter_context(tc.tile_pool(name="const", bufs=1))
        spool = ctx.enter_context(tc.tile_pool(name="sbuf", bufs=3))
        wpool = ctx.enter_context(tc.tile_pool(name="wsel", bufs=2))
        gpool = ctx.enter_context(tc.tile_pool(name="gath", bufs=3))
        ppool = ctx.enter_context(tc.tile_pool(name="psum", bufs=2,
                                               space="PSUM"))
        ppool2 = ctx.enter_context(tc.tile_pool(name="psum2", bufs=2,
                                                space="PSUM"))
        pagg = ctx.enter_context(tc.tile_pool(name="pagg", bufs=2,
                                              space="PSUM"))
        pacc = ctx.enter_context(tc.tile_pool(name="pacc", bufs=1,
                                              space="PSUM"))
        dpool = ctx.enter_context(tc.tile_pool(name="dram", bufs=1,
                                               space="DRAM"))

        # ---- constants -----------------------------------------------------
        ident = cpool.tile([128, 128], F32, tag="ident")
        make_identity(nc, ident[:])
        ones_row = cpool.tile([1, 128], F32, tag="ones_row")
        nc.vector.memset(ones_row[:], 1.0)

        def wtiles(nm, rows, cols):
            ts = []
            for i in range(0, rows, 128):
                p = min(128, rows - i)
                t = cpool.tile([p, cols], F32, tag=f"w_{nm}_{i}")
                nc.sync.dma_start(out=t[:], in_=wdram[nm][i:i + p, :])
                ts.append(t)
            return ts

        ev_ia = cpool.tile([128, n_blk * CA * 8], I16, tag="ev_ia")
        nc.sync.dma_start(out=ev_ia[:], in_=ev_idx_a[:])
        ev_ib = cpool.tile([128, n_blk * CB * 8], I16, tag="ev_ib")
        nc.sync.dma_start(out=ev_ib[:], in_=ev_idx_b[:])
        du_ia = cpool.tile([128, n_blk * CDA * 8], I16, tag="du_ia")
        nc.sync.dma_start(out=du_ia[:], in_=du_idx_a[:])
        du_ib = cpool.tile([128, n_blk * CDB * 8], I16, tag="du_ib")
        nc.sync.dma_start(out=du_ib[:], in_=du_idx_b[:])
        ssel = cpool.tile([128, n_blk * B], F32, tag="ssel")
        nc.sync.dma_start(out=ssel[:], in_=ssel_in[:])

        W = {}
        for nm, sh in wnames:
            W[nm] = wtiles(nm, sh[0], sh[1])

        def load_wsel(wsel, wsel_dram, c0, nchunk):
            """Stream nchunk host-built one-hot chunks from DRAM via the
            Activation engine's HWDGE queue (avoids the Sync DMA FIFO that
            carries prevT/output traffic)."""
            nc.scalar.dma_start(
                out=wsel[:, :nchunk * 128],
                in_=wsel_dram[:, c0 * 128:(c0 + nchunk) * 128])

        # fold c1_Wroot + skip_W (both multiply xcT in c1 stage2)
        W_rs = []
        for kh in range(2):
            t = cpool.tile([128, 256], F32, tag=f"w_rs_{kh}")
            nc.vector.tensor_add(out=t[:], in0=W["c1_Wroot"][kh][:],
                                 in1=W["skip_W"][kh][:])
            W_rs.append(t)

        # ---- DRAM intermediates -------------------------------------------
        sp = cfg["SHARD_PAD"]
        h1T_local = dpool.tile([128, sp], F32, tag="h1T_local")
        h1b_local = dpool.tile([sp, 128], BF16, tag="h1b_local")
        h1_full = dpool.tile([npad, 128], BF16, tag="h1_full",
                             addr_space="Shared")
        # combined [h2|d] bf16 table (g2 writes cols 0:128, d1 cols 128:256)
        hd_local = dpool.tile([sp, 256], F8, tag="hd_local")
        hd_full = dpool.tile([npad, 256], F8, tag="hd_full",
                             addr_space="Shared")
        h2T_local = dpool.tile([128, sp], F32, tag="h2T_local")
        dT_local = dpool.tile([128, sp], F32, tag="dT_local")
        ar_in = dpool.tile([B, 257], F32, tag="ar_in")
        ar_out = dpool.tile([B, 257], F32, tag="ar_out", addr_space="Shared")

        # ---- generic GraphConv layer (F_out = 128) ------------------------
        def g_layer(lname, fin, gfin, tbl, idx_a, idx_b, ca, cb, wsel_dram,
                    prevT_dram, wr, wroot, br, outT_dram, out_bf, out_bf_col,
                    npair=4):
            c = ca + cb
            for bp in range(0, n_blk, npair):
                np_ = min(npair, n_blk - bp)
                tot = np_ * c
                gath = gpool.tile([128, tot * gfin], BF16, tag="gath")
                for (qn, (idx, cc, off, th)) in enumerate((
                    (idx_a, ca, 0, tbl[0]),
                    (idx_b, cb, np_ * ca, tbl[1]),
                )):
                    span = np_ * cc
                    lo = span // 2
                    for (sq, c0, c1) in ((qn, 0, lo), (qn + 2, lo, span)):
                        if c1 == c0:
                            continue
                        nc.gpsimd.dma_gather(
                            out_ap=gath[:, (off + c0) * gfin:
                                        (off + c1) * gfin].rearrange(
                                "p (c f) -> p c f", c=c1 - c0),
                            in_ap=th,
                            idxs_ap=idx[:, bp * cc * 8 + c0 * 8:
                                        bp * cc * 8 + c1 * 8],
                            num_idxs=(c1 - c0) * 128,
                            num_idxs_reg=(c1 - c0) * 128,
                            elem_size=gfin,
                            single_packet=False,
                            queue_num=sq,
                        )
                gmm = gath

                wsel = wpool.tile([128, np_ * c * 128], BF16, tag="wsel")
                load_wsel(wsel, wsel_dram, bp * c, np_ * c)

                for r in range(np_):
                    b = bp + r

                    def gpos(j, r=r):
                        if j < ca:
                            return r * ca + j
                        return np_ * ca + r * cb + (j - ca)

                    agg_ps = pagg.tile([fin, 128], F32, tag="agg_ps")
                    for j in range(c):
                        g0 = gpos(j) * gfin
                        nc.tensor.matmul(
                            out=agg_ps[:],
                            lhsT=gmm[:, g0:g0 + fin],
                            rhs=wsel[:, (r * c + j) * 128:
                                     (r * c + j + 1) * 128],
                            start=(j == 0), stop=(j == c - 1))
                    agg = spool.tile([fin, 128], F32, tag="agg_sb")
                    nc.scalar.copy(agg[:], agg_ps[:])

                    prevT = spool.tile([fin, 128], F32, tag="prevT")
                    nc.sync.dma_start(
                        out=prevT[:],
                        in_=prevT_dram[:, b * 128:(b + 1) * 128])

                    o_ps = ppool2.tile([128, 128], F32, tag="o")
                    nc.tensor.matmul(out=o_ps[:], lhsT=wr[0][:], rhs=agg[:],
                                     start=True, stop=False)
                    nc.tensor.matmul(out=o_ps[:], lhsT=wroot[0][:],
                                     rhs=prevT[:], start=False, stop=True)
                    oT = spool.tile([128, 128], F32, tag="oT_sb")
                    nc.scalar.activation(out=oT[:], in_=o_ps[:], func=AF.Relu,
                                         bias=br[0][:, :1], scale=1.0)
                    nc.sync.dma_start(
                        out=outT_dram[:, b * 128:(b + 1) * 128], in_=oT[:])
                    # node-major bf16 copy for the next gather table
                    nm_ps = ppool.tile([128, 128], F32, tag="tp")
                    nc.tensor.transpose(out=nm_ps[:], in_=oT[:],
                                        identity=ident[:])
                    obf = spool.tile([128, 128], out_bf.dtype,
                                     tag="obf_sb")
                    nc.scalar.copy(obf[:], nm_ps[:])
                    nc.sync.dma_start(
                        out=out_bf[b * 128:(b + 1) * 128,
                                   out_bf_col:out_bf_col + 128],
                        in_=obf[:])

        # ---- g1 ------------------------------------------------------------
        g_layer("g1", 128, 128, (x_pad[0:half, :], x_pad[half:npad, :]),
                ev_ia, ev_ib, CA, CB, ev_wsel16,
                x_localT, W["g1_Wr"], W["g1_Wroot"], W["g1_br"],
                h1T_local, h1b_local, 0)
        nc.gpsimd.collective_compute(
            "AllGather", OP.bypass, replica_groups=[list(range(NC))],
            ins=[h1b_local.opt()], outs=[h1_full.opt()])

        # ---- d1 (independent of h1; overlaps the AllGather) ---------------
        g_layer("d1", 64, 128, (dur_pad[0:half, :], dur_pad[half:npad, :]),
                du_ia, du_ib, CDA, CDB, du_wsel16,
                dur_localT, W["d1_Wr"], W["d1_Wroot"], W["d1_br"],
                dT_local, hd_local, 128)

        # ---- g2 ------------------------------------------------------------
        g_layer("g2", 128, 128, (h1_full[0:half, :], h1_full[half:npad, :]),
                ev_ia, ev_ib, CA, CB, ev_wsel16,
                h1T_local, W["g2_Wr"], W["g2_Wroot"], W["g2_br"],
                h2T_local, hd_local, 0)

        nc.gpsimd.collective_compute(
            "AllGather", OP.bypass, replica_groups=[list(range(NC))],
            ins=[hd_local.opt()], outs=[hd_full.opt()])

        # ---- c1 + pooling --------------------------------------------------
        pooled = cpool.tile([B, 257], F32, tag="pooled")
        nc.vector.memset(pooled[:], 0.0)
        tbl = (hd_full[0:half, :], hd_full[half:npad, :])
        for bp in range(0, n_blk, 2):
            np_ = min(2, n_blk - bp)
            ctot = np_ * C
            gath = gpool.tile([128, ctot * 256], F8, tag="gath")
            for (qn, (idx, cc, off, th)) in enumerate((
                (ev_ia, CA, 0, tbl[0]),
                (ev_ib, CB, np_ * CA, tbl[1]),
            )):
                span = np_ * cc
                lo = span // 2
                for (sq, c0, c1) in ((qn, 0, lo), (qn + 2, lo, span)):
                    if c1 == c0:
                        continue
                    nc.gpsimd.dma_gather(
                        out_ap=gath[:, (off + c0) * 256:
                                    (off + c1) * 256].rearrange(
                            "p (c f) -> p c f", c=c1 - c0),
                        in_ap=th,
                        idxs_ap=idx[:, bp * cc * 8 + c0 * 8:
                                    bp * cc * 8 + c1 * 8],
                        num_idxs=(c1 - c0) * 128,
                        num_idxs_reg=(c1 - c0) * 128,
                        elem_size=256,
                        single_packet=False,
                        queue_num=sq,
                    )
            wsel = wpool.tile([128, np_ * C * 128], F8, tag="wsel")
            nc.scalar.dma_start(
                out=wsel[:, :np_ * C * 128],
                in_=ev_wsel8[:, bp * C * 128:(bp * C + np_ * C) * 128])

            for r in range(np_):
                b = bp + r

                def gpos_c1(j, r=r, np_=np_):
                    if j < CA:
                        return r * CA + j
                    return np_ * CA + r * CB + (j - CA)

                agg_f_ps = pacc.tile([128, 128], F32, tag="agg_f")
                agg_d_ps = pacc.tile([128, 128], F32, tag="agg_d")
                for j in range(C):
                    g0 = gpos_c1(j) * 256
                    ws = wsel[:, (r * C + j) * 128:(r * C + j + 1) * 128]
                    nc.tensor.matmul(
                        out=agg_f_ps[:], lhsT=gath[:, g0:g0 + 128],
                        rhs=ws, start=(j == 0), stop=(j == C - 1))
                    nc.tensor.matmul(
                        out=agg_d_ps[:], lhsT=gath[:, g0 + 128:g0 + 256],
                        rhs=ws, start=(j == 0), stop=(j == C - 1))
                agg_f = spool.tile([128, 128], F32, tag="aggc_f")
                nc.scalar.copy(agg_f[:], agg_f_ps[:])
                agg_d = spool.tile([128, 128], F32, tag="aggc_d")
                nc.scalar.copy(agg_d[:], agg_d_ps[:])
                aggs = (agg_f, agg_d)

                xcT = []
                for kh, src_dram in ((0, h2T_local), (1, dT_local)):
                    t_sb = spool.tile([128, 128], F32, tag=f"xcT_sb{kh}")
                    nc.sync.dma_start(
                        out=t_sb[:],
                        in_=src_dram[:, b * 128:(b + 1) * 128])
                    xcT.append(t_sb)

                # node-major y: out[dst, fout] accumulated in one PSUM tile;
                # bias added via a rank-1 ones-row matmul, so relu needs no
                # per-partition bias
                y_ps = ppool2.tile([128, 256], F32, tag="o")
                nc.tensor.matmul(out=y_ps[:], lhsT=ones_row[:1, :],
                                 rhs=W["bias_c_row"][0][:1, :],
                                 start=True, stop=False)
                for kh in range(2):
                    nc.tensor.matmul(out=y_ps[:], lhsT=aggs[kh][:],
                                     rhs=W["c1_Wr"][kh][:], start=False,
                                     stop=False)
                    nc.tensor.matmul(out=y_ps[:], lhsT=xcT[kh][:],
                                     rhs=W_rs[kh][:], start=False,
                                     stop=(kh == 1))
                y_nm = spool.tile([128, 272], F32, tag="y_nm")
                nc.vector.memset(y_nm[:, 256:257], 1.0)
                nc.scalar.activation(out=y_nm[:, :256], in_=y_ps[:],
                                     func=AF.Relu)
                pool_ps = ppool2.tile([B, 257], F32, tag="o")
                nc.tensor.matmul(out=pool_ps[:],
                                 lhsT=ssel[:, b * B:(b + 1) * B],
                                 rhs=y_nm[:, :257], start=True, stop=True)
                nc.vector.tensor_add(out=pooled[:], in0=pooled[:],
                                     in1=pool_ps[:])

        # ---- AllReduce pooled sums + counts -------------------------------
        nc.sync.dma_start(out=ar_in[:], in_=pooled[:])
        nc.gpsimd.collective_compute(
            "AllReduce", OP.add, replica_groups=[list(range(NC))],
            ins=[ar_in.opt()], outs=[ar_out.opt()])

        # ---- head (replicated on every core) ------------------------------
        pl = spool.tile([B, 257], F32, tag="pl")
        nc.sync.dma_start(out=pl[:], in_=ar_out[:])
        cnt = spool.tile([B, 1], F32, tag="cnt")
        nc.vector.tensor_scalar(out=cnt[:], in0=pl[:, 256:257], scalar1=1.0,
                                scalar2=None, op0=OP.max)
        rec = spool.tile([B, 1], F32, tag="rec")
        nc.vector.reciprocal(out=rec[:], in_=cnt[:])
        emb = spool.tile([B, 256], F32, tag="emb")
        nc.vector.tensor_tensor(out=emb[:], in0=pl[:, :256],
                                in1=rec[:, :1].to_broadcast([B, 256]),
                                op=OP.mult)

        def transpose_2(src, tag):
            ts = []
            for kh in range(2):
                t_ps = ppool.tile([128, B], F32, tag="tp")
                nc.tensor.transpose(out=t_ps[:],
                                    in_=src[:, kh * 128:(kh + 1) * 128],
                                    identity=ident[:B, :B])
                t_sb = spool.tile([128, B], F32, tag=f"{tag}_sb{kh}")
                nc.vector.tensor_copy(t_sb[:], t_ps[:])
                ts.append(t_sb)
            return ts

        embT = transpose_2(emb, "embT")

        seq = spool.tile([B, 256], F32, tag="seq")
        nc.sync.dma_start(out=seq[:], in_=seq_in[:])
        seqT = transpose_2(seq, "seqT")

        def mlp(rhss, wname, bname, act, out_halves, tag):
            outs = []
            for o in range(out_halves):
                osl = slice(o * 128, (o + 1) * 128)
                ps = ppool2.tile([128, B], F32, tag="o")
                for si, r in enumerate(rhss):
                    nc.tensor.matmul(out=ps[:], lhsT=W[wname][si][:, osl],
                                     rhs=r[:], start=(si == 0),
                                     stop=(si == len(rhss) - 1))
                t = spool.tile([128, B], F32, tag=f"{tag}_sb{o}")
                nc.scalar.activation(out=t[:], in_=ps[:], func=act,
                                     bias=W[bname][o][:, :1], scale=1.0)
                outs.append(t)
            return outs

        s1T = mlp(seqT, "fc1_W", "fc1_b", AF.Relu, 2, "s1")
        sT = mlp(s1T, "fc2_W", "fc2_b", AF.Relu, 1, "s2")
        hT = mlp(embT + sT, "fcc_W", "fcc_b", AF.Relu, 2, "hc")

        lg_ps = ppool2.tile([B, 16], F32, tag="o")
        for o in range(2):
            nc.tensor.matmul(out=lg_ps[:], lhsT=hT[o][:], rhs=W["cls_W"][o][:],
                             start=(o == 0), stop=(o == 1))
        logits = spool.tile([B, 16], F32, tag="logits")
        nc.vector.tensor_tensor(out=logits[:], in0=lg_ps[:],
                                in1=W["cls_b_rep"][0][:], op=OP.add)
        rmax = spool.tile([B, 1], F32, tag="rmax")
        nc.vector.tensor_reduce(out=rmax[:], in_=logits[:],
                                axis=mybir.AxisListType.X, op=OP.max)
        tshift = spool.tile([B, 16], F32, tag="tshift")
        nc.vector.tensor_scalar(out=tshift[:], in0=logits[:],
                                scalar1=rmax[:, :1], scalar2=None,
                                op0=OP.subtract)
        ex = spool.tile([B, 16], F32, tag="ex")
        nc.scalar.activation(out=ex[:], in_=tshift[:], func=AF.Exp)
        esum = spool.tile([B, 1], F32, tag="esum")
        nc.vector.tensor_reduce(out=esum[:], in_=ex[:],
                                axis=mybir.AxisListType.X, op=OP.add)
        lsum = spool.tile([B, 1], F32, tag="lsum")
        nc.scalar.activation(out=lsum[:], in_=esum[:], func=AF.Ln)
        res = spool.tile([B, 16], F32, tag="res")
        nc.vector.tensor_scalar(out=res[:], in0=tshift[:],
                                scalar1=lsum[:, :1], scalar2=None,
                                op0=OP.subtract)
        nc.sync.dma_start(out=out_ext[:], in_=res[:])

    nc.compile()
    return nc


# --------------------------------------------------------------------------
# Host orchestration
# --------------------------------------------------------------------------

def make_in_maps(inputs, cfg):
    import ml_dtypes
    x = np.asarray(inputs["x"], np.float32)
    # Reference masks x at -1.0 sentinels (and the post-layer masks are
    # no-ops given relu(-1.0) == 0), so pre-mask on host once.
    x = np.where(x == -1.0, 0.0, x)
    dur_x = np.asarray(inputs["dur_x"], np.float32)
    batch = np.asarray(inputs["batch"], np.int64)

    ev_planes, CA, CB = prep_edges(inputs["edge_index"], inputs["edge_attr"],
                                   cfg)
    du_planes, CDA, CDB = prep_edges(inputs["dur_edge_index"],
                                     inputs["dur_edge_attr"], cfg)

    x_pad_f32 = _pad_nodes(x, cfg)
    x_pad = x_pad_f32.astype(ml_dtypes.bfloat16)
    dur_padded = _pad_nodes(dur_x, cfg)
    # bf16 dur table padded to 128 features (256B rows for the gather)
    dur_pad_bf16 = np.zeros((dur_padded.shape[0], 128), ml_dtypes.bfloat16)
    dur_pad_bf16[:, :64] = dur_padded.astype(ml_dtypes.bfloat16)

    n_blk = cfg["SHARD_PAD"] // 128
    B = cfg["B"]
    bias_c = (np.asarray(inputs["c1_br"], np.float32)
              + np.asarray(inputs["skip_b"], np.float32))

    def col(v):
        return np.ascontiguousarray(
            np.asarray(v, np.float32).reshape(-1, 1))

    weights = dict(
        g1_Wr=inputs["g1_Wr"], g1_br=col(inputs["g1_br"]),
        g1_Wroot=inputs["g1_Wroot"],
        g2_Wr=inputs["g2_Wr"], g2_br=col(inputs["g2_br"]),
        g2_Wroot=inputs["g2_Wroot"],
        d1_Wr=inputs["d1_Wr"], d1_br=col(inputs["d1_br"]),
        d1_Wroot=inputs["d1_Wroot"],
        c1_Wr=inputs["c1_Wr"], c1_Wroot=inputs["c1_Wroot"],
        skip_W=inputs["skip_W"],
        bias_c_row=np.asarray(bias_c, np.float32).reshape(1, -1),
        fc1_W=inputs["fc1_W"], fc1_b=col(inputs["fc1_b"]),
        fc2_W=inputs["fc2_W"], fc2_b=col(inputs["fc2_b"]),
        fcc_W=inputs["fcc_W"], fcc_b=col(inputs["fcc_b"]),
        cls_W=inputs["cls_W"],
        cls_b_rep=np.tile(np.asarray(inputs["cls_b"], np.float32)[None, :],
                          (B, 1)),
        seq_features=inputs["seq_features"],
    )
    weights = {k: np.ascontiguousarray(np.asarray(v, np.float32))
               for k, v in weights.items()}

    in_maps = []
    for k in range(NC):
        sp = cfg["SHARD_PAD"]
        # graph-membership one-hot [128 node-in-block, n_blk * B]
        bfr_flat = np.full(sp, -1, np.int64)
        bfr_flat[:cfg["SHARD"]] = batch[k * cfg["SHARD"]:(k + 1) * cfg["SHARD"]]
        ssel = np.zeros((n_blk, 128, B), np.float32)
        bb = bfr_flat.reshape(n_blk, 128)
        blk_i, pos_i = np.nonzero(bb >= 0)
        ssel[blk_i, pos_i, bb[blk_i, pos_i]] = 1.0
        ssel = np.ascontiguousarray(
            ssel.transpose(1, 0, 2).reshape(128, n_blk * B))

        m = dict(
            x_pad=x_pad,
            x_localT=np.ascontiguousarray(
                x_pad_f32[k * sp:(k + 1) * sp].T),
            dur_pad=dur_pad_bf16,
            dur_localT=np.ascontiguousarray(
                dur_padded[k * sp:(k + 1) * sp].T),
            ev_idx_a=ev_planes[k]["idx_a"], ev_idx_b=ev_planes[k]["idx_b"],
            ev_wsel16=ev_planes[k]["wsel16"], ev_wsel8=ev_planes[k]["wsel8"],
            du_idx_a=du_planes[k]["idx_a"], du_idx_b=du_planes[k]["idx_b"],
            du_wsel16=du_planes[k]["wsel16"],
            ssel=ssel,
            **weights,
        )
        in_maps.append(m)
    return in_maps, (CA, CB, CDA, CDB)


_LAST_RESULT = None


def kernel(**inputs) -> np.ndarray:
    global _LAST_RESULT
    cfg = dict(REAL)
    cfg["N"] = inputs["x"].shape[0]
    cfg["B"] = inputs["seq_features"].shape[0]
    in_maps, (CA, CB, CDA, CDB) = make_in_maps(inputs, cfg)
    nc = build_program(cfg, CA, CB, CDA, CDB)
    from concourse.bass_utils import run_bass_kernel_spmd
    res = run_bass_kernel_spmd(nc, in_maps, list(range(NC)))
    _LAST_RESULT = res
    return np.asarray(res.results[0]["out"], np.float32)



# revision 35
# speedup vs baseline: 1.1481x; 1.1222x over previous
"""Trainium2 Bass kernel for nn_EventSequenceDurationGraphConvModel.

Self-contained: accepts FULL inputs, shards across 8 NeuronCores internally
(nodes/edges partitioned by destination node per core), runs one SPMD Bass
program, and returns the FULL [64, 16] output.

Per-core GraphConv layers aggregate via dma_gather of source rows (bf16)
followed by PSUM matmuls against one-hot selection matrices
W_sel[e, d] = ew[e] * (dst_rel[e] == d), so the segment_sum needs no
scatter. Key performance structure:
  - SWDGE descriptor dispatch is the machine bottleneck (~5ns/descriptor,
    one descriptor per gathered edge row). Gathers are spread across 4
    SWDGE queues (ucode max), which roughly halves effective dispatch
    time vs a single queue. single_packet must stay False (True crashes).
  - W_sel matrices are built on-device with TWO broadcast tensor_tensor
    mega-ops per block group (is_equal against an iota row, then in-place
    multiply by edge weights) -- cheap on DVE and no HBM traffic to
    contend with gather descriptor dispatch.
  - c1 gathers ONE combined [h2|d] bf16 table with 512B rows (one
    descriptor per edge instead of two), saving ~20% of all descriptors.
  - dur table is bf16 padded to 128 features so d1's gather needs no
    f32 download + cast.
  - Root-term inputs stay feature-major ([F, nodes]); c1's output stage
    computes node-major y directly (swapped matmul operands, bias folded
    in via a rank-1 ones-row matmul), so no PE transposes there.
  - Host pre-masks x (x == -1.0 -> 0); the reference's post-layer mask
    ops are no-ops given relu(-1.0) == 0, so no device masking at all.
  - gather tiles are triple-buffered so descriptor generation for group
    n+2 overlaps compute of group n.

Pipeline per core (fp32 accumulation, bf16 gathers/matmuls):
  g1 -> AllGather(h1) overlapped with d1 -> g2 -> AllGather([h2|d]) -> c1
  -> pool (PSUM matmul against host-built one-hot graph membership)
  -> AllReduce -> replicated MLP head + log_softmax.
"""
import sys

import numpy as np

sys.path.insert(0, "/opt/trn_rl_repo")

from concourse import bacc, bass, mybir  # noqa: E402
import concourse.tile as tile  # noqa: E402
from concourse.masks import make_identity  # noqa: E402

F32 = mybir.dt.float32
BF16 = mybir.dt.bfloat16
F8 = mybir.dt.float8e4
I16 = mybir.dt.int16
AF = mybir.ActivationFunctionType
OP = mybir.AluOpType

NC = 8

REAL = dict(N=50000, E=800000, B=64, SHARD=6250, SHARD_PAD=6272)


# --------------------------------------------------------------------------
# Host-side sharding / preprocessing (pure index/layout work)
# --------------------------------------------------------------------------

def _gpid(node_id, cfg):
    """Real node id -> padded global id."""
    return (node_id // cfg["SHARD"]) * cfg["SHARD_PAD"] + node_id % cfg["SHARD"]


def _wrap_idx(flat_i16):
    """Flat int16 index list -> dma_gather plane [128, n/16] (16-part wrap,
    replicated across the 8 gpsimd cores)."""
    n = flat_i16.shape[0]
    assert n % 16 == 0
    return np.tile(flat_i16.reshape(n // 16, 16).T, (8, 1)).copy()


def prep_edges(edge_index, edge_attr, cfg):
    """Shard + sort + pad the edge list. Returns per-core gather planes,
    host-built W_sel planes, and uniform per-block chunk counts (CA, CB)."""
    import ml_dtypes
    n_blk = cfg["SHARD_PAD"] // 128
    half = NC * cfg["SHARD_PAD"] // 2
    src = np.asarray(edge_index[0], dtype=np.int64)
    dst = np.asarray(edge_index[1], dtype=np.int64)
    ew = np.asarray(edge_attr, dtype=np.float32)
    gsrc = _gpid(src, cfg)
    core = dst // cfg["SHARD"]
    dloc = dst % cfg["SHARD"]

    per_core = []
    ca_max = cb_max = 1
    for k in range(NC):
        sel = np.nonzero(core == k)[0]
        order = sel[np.argsort(dloc[sel], kind="stable")]
        gs, dl, w = gsrc[order], dloc[order], ew[order]
        blk = dl // 128
        rel = dl % 128
        blocks = []
        for b in range(n_blk):
            m = blk == b
            in_a = gs[m] < half
            a = (gs[m][in_a], rel[m][in_a], w[m][in_a])
            bb = (gs[m][~in_a] - half, rel[m][~in_a], w[m][~in_a])
            blocks.append((a, bb))
            ca_max = max(ca_max, -(-len(a[0]) // 128))
            cb_max = max(cb_max, -(-len(bb[0]) // 128))
        per_core.append(blocks)

    CA, CB = ca_max, cb_max
    C = CA + CB
    planes = []
    for k in range(NC):
        idx_a = np.zeros(n_blk * CA * 128, np.int16)
        idx_b = np.zeros(n_blk * CB * 128, np.int16)
        # compact per-edge planes: dst-relative id + edge weight, laid out
        # [128 edge-pos, n_blk*C chunks] (pad edges get ew=0)
        dstf = np.zeros((n_blk * C, 128), np.float32)
        ewf = np.zeros((n_blk * C, 128), np.float32)
        dsti = np.zeros((n_blk * C, 128), np.int64)
        for b, (a, bb) in enumerate(per_core[k]):
            na, nb = len(a[0]), len(bb[0])
            idx_a[b * CA * 128:b * CA * 128 + na] = a[0].astype(np.int16)
            idx_b[b * CB * 128:b * CB * 128 + nb] = bb[0].astype(np.int16)
            for (cnt, off, rels, ws) in ((na, 0, a[1], a[2]),
                                         (nb, CA, bb[1], bb[2])):
                if cnt == 0:
                    continue
                e = np.arange(cnt)
                chunk = b * C + off + e // 128
                dstf[chunk, e % 128] = rels
                ewf[chunk, e % 128] = ws
                dsti[chunk, e % 128] = rels.astype(np.int64)
        # host-built fp8 one-hot W_sel plane for c1 (streamed from DRAM
        # on device; c1's on-device fp8 DVE build was phase-limiting)
        tot = n_blk * C
        W = np.zeros((tot, 128, 128), ml_dtypes.float8_e4m3)
        ch = np.arange(tot)[:, None]
        ee = np.arange(128)[None, :]
        W[ch, ee, dsti] = ewf.astype(ml_dtypes.bfloat16).astype(
            ml_dtypes.float8_e4m3)
        planes.append(dict(
            idx_a=_wrap_idx(idx_a),
            idx_b=_wrap_idx(idx_b),
            dstf=np.ascontiguousarray(dstf.T).astype(ml_dtypes.bfloat16),
            ewf=np.ascontiguousarray(ewf.T).astype(ml_dtypes.bfloat16),
            wsel8=np.ascontiguousarray(
                W.transpose(1, 0, 2).reshape(128, tot * 128)),
        ))
    return planes, CA, CB


def _pad_nodes(arr, cfg):
    """[N, F] -> [NC*SHARD_PAD, F] with zero-filled pad rows per shard."""
    f = arr.shape[1]
    out = np.zeros((NC * cfg["SHARD_PAD"], f), arr.dtype)
    for k in range(NC):
        out[k * cfg["SHARD_PAD"]:k * cfg["SHARD_PAD"] + cfg["SHARD"]] = (
            arr[k * cfg["SHARD"]:(k + 1) * cfg["SHARD"]]
        )
    return out


# --------------------------------------------------------------------------
# Device program
# --------------------------------------------------------------------------

def build_program(cfg, CA, CB, CDA, CDB):
    n_blk = cfg["SHARD_PAD"] // 128
    npad = NC * cfg["SHARD_PAD"]
    half = npad // 2
    B = cfg["B"]
    C = CA + CB
    CD = CDA + CDB

    nc = bacc.Bacc("TRN2", target_bir_lowering=False, debug=False,
                   num_devices=NC, num_swdge_queues=4)

    def din(name, shape, dt=F32):
        return nc.declare_dram_parameter(name, list(shape), dt, isOutput=False)

    x_pad = din("x_pad", [npad, 128], BF16)
    x_localT = din("x_localT", [128, cfg["SHARD_PAD"]])
    dur_pad = din("dur_pad", [npad, 128], BF16)
    dur_localT = din("dur_localT", [64, cfg["SHARD_PAD"]])
    ev_idx_a = din("ev_idx_a", [128, n_blk * CA * 8], I16)
    ev_idx_b = din("ev_idx_b", [128, n_blk * CB * 8], I16)
    ev_dstf = din("ev_dstf", [128, n_blk * C], BF16)
    ev_wsel8 = din("ev_wsel8", [128, n_blk * C * 128], F8)
    ev_ewf = din("ev_ewf", [128, n_blk * C], BF16)
    du_idx_a = din("du_idx_a", [128, n_blk * CDA * 8], I16)
    du_idx_b = din("du_idx_b", [128, n_blk * CDB * 8], I16)
    du_dstf = din("du_dstf", [128, n_blk * CD], BF16)
    du_ewf = din("du_ewf", [128, n_blk * CD], BF16)
    ssel_in = din("ssel", [128, n_blk * B])
    seq_in = din("seq_features", [B, 256])

    wnames = [
        ("g1_Wr", [128, 128]), ("g1_br", [128, 1]), ("g1_Wroot", [128, 128]),
        ("g2_Wr", [128, 128]), ("g2_br", [128, 1]), ("g2_Wroot", [128, 128]),
        ("d1_Wr", [64, 128]), ("d1_br", [128, 1]), ("d1_Wroot", [64, 128]),
        ("c1_Wr", [256, 256]), ("c1_Wroot", [256, 256]),
        ("skip_W", [256, 256]), ("bias_c_row", [1, 256]),
        ("fc1_W", [256, 256]), ("fc1_b", [256, 1]),
        ("fc2_W", [256, 128]), ("fc2_b", [128, 1]),
        ("fcc_W", [384, 256]), ("fcc_b", [256, 1]),
        ("cls_W", [256, 16]), ("cls_b_rep", [B, 16]),
    ]
    wdram = {nm: din(nm, sh) for nm, sh in wnames}
    out_ext = nc.declare_dram_parameter("out", [B, 16], F32, isOutput=True)

    from contextlib import ExitStack
    with tile.TileContext(nc) as tc, ExitStack() as ctx:
        cpool = ctx.enter_context(tc.tile_pool(name="const", bufs=1))
        spool = ctx.enter_context(tc.tile_pool(name="sbuf", bufs=3))
        wpool = ctx.enter_context(tc.tile_pool(name="wsel", bufs=2))
        gpool = ctx.enter_context(tc.tile_pool(name="gath", bufs=3))
        ppool = ctx.enter_context(tc.tile_pool(name="psum", bufs=2,
                                               space="PSUM"))
        ppool2 = ctx.enter_context(tc.tile_pool(name="psum2", bufs=2,
                                                space="PSUM"))
        pagg = ctx.enter_context(tc.tile_pool(name="pagg", bufs=2,
                                              space="PSUM"))
        pacc = ctx.enter_context(tc.tile_pool(name="pacc", bufs=1,
                                              space="PSUM"))
        dpool = ctx.enter_context(tc.tile_pool(name="dram", bufs=1,
                                               space="DRAM"))

        # ---- constants -----------------------------------------------------
        ident = cpool.tile([128, 128], F32, tag="ident")
        make_identity(nc, ident[:])
        iota_i = cpool.tile([128, 128], mybir.dt.int32, tag="iota_i")
        nc.gpsimd.iota(iota_i[:], pattern=[[1, 128]], base=0,
                       channel_multiplier=0)
        iotab = cpool.tile([128, 128], BF16, tag="iotab")
        nc.vector.tensor_copy(iotab[:], iota_i[:])
        ones_row = cpool.tile([1, 128], F32, tag="ones_row")
        nc.vector.memset(ones_row[:], 1.0)

        def wtiles(nm, rows, cols):
            ts = []
            for i in range(0, rows, 128):
                p = min(128, rows - i)
                t = cpool.tile([p, cols], F32, tag=f"w_{nm}_{i}")
                nc.sync.dma_start(out=t[:], in_=wdram[nm][i:i + p, :])
                ts.append(t)
            return ts

        ev_ia = cpool.tile([128, n_blk * CA * 8], I16, tag="ev_ia")
        nc.sync.dma_start(out=ev_ia[:], in_=ev_idx_a[:])
        ev_ib = cpool.tile([128, n_blk * CB * 8], I16, tag="ev_ib")
        nc.sync.dma_start(out=ev_ib[:], in_=ev_idx_b[:])
        du_ia = cpool.tile([128, n_blk * CDA * 8], I16, tag="du_ia")
        nc.sync.dma_start(out=du_ia[:], in_=du_idx_a[:])
        du_ib = cpool.tile([128, n_blk * CDB * 8], I16, tag="du_ib")
        nc.sync.dma_start(out=du_ib[:], in_=du_idx_b[:])
        ev_d = cpool.tile([128, n_blk * C], BF16, tag="ev_d")
        nc.sync.dma_start(out=ev_d[:], in_=ev_dstf[:])
        ev_w = cpool.tile([128, n_blk * C], BF16, tag="ev_w")
        nc.sync.dma_start(out=ev_w[:], in_=ev_ewf[:])
        du_d = cpool.tile([128, n_blk * CD], BF16, tag="du_d")
        nc.sync.dma_start(out=du_d[:], in_=du_dstf[:])
        du_w = cpool.tile([128, n_blk * CD], BF16, tag="du_w")
        nc.sync.dma_start(out=du_w[:], in_=du_ewf[:])
        ssel = cpool.tile([128, n_blk * B], F32, tag="ssel")
        nc.sync.dma_start(out=ssel[:], in_=ssel_in[:])

        W = {}
        for nm, sh in wnames:
            W[nm] = wtiles(nm, sh[0], sh[1])

        def build_wsel(wsel, dstf, ewf, c0, nchunk):
            """wsel[:, j*128:(j+1)*128] = (dstf[:, c0+j] == iota) * ewf[:, c0+j]
            for all nchunk chunks in two broadcast tensor_tensor mega-ops."""
            out3 = wsel[:, :nchunk * 128].rearrange("p (c f) -> p c f",
                                                    c=nchunk)
            it3 = iotab[:].rearrange("(p u) f -> p u f", u=1).to_broadcast(
                [128, nchunk, 128])
            nc.vector.tensor_tensor(
                out=out3,
                in0=dstf[:, c0:c0 + nchunk].rearrange(
                    "p (c u) -> p c u", u=1).to_broadcast([128, nchunk, 128]),
                in1=it3, op=OP.is_equal)
            nc.vector.tensor_tensor(
                out=out3,
                in0=ewf[:, c0:c0 + nchunk].rearrange(
                    "p (c u) -> p c u", u=1).to_broadcast([128, nchunk, 128]),
                in1=out3, op=OP.mult)

        # fold c1_Wroot + skip_W (both multiply xcT in c1 stage2)
        W_rs = []
        for kh in range(2):
            t = cpool.tile([128, 256], F32, tag=f"w_rs_{kh}")
            nc.vector.tensor_add(out=t[:], in0=W["c1_Wroot"][kh][:],
                                 in1=W["skip_W"][kh][:])
            W_rs.append(t)

        # ---- DRAM intermediates -------------------------------------------
        sp = cfg["SHARD_PAD"]
        h1T_local = dpool.tile([128, sp], F32, tag="h1T_local")
        h1b_local = dpool.tile([sp, 128], BF16, tag="h1b_local")
        h1_full = dpool.tile([npad, 128], BF16, tag="h1_full",
                             addr_space="Shared")
        # combined [h2|d] bf16 table (g2 writes cols 0:128, d1 cols 128:256)
        hd_local = dpool.tile([sp, 256], F8, tag="hd_local")
        hd_full = dpool.tile([npad, 256], F8, tag="hd_full",
                             addr_space="Shared")
        h2T_local = dpool.tile([128, sp], F32, tag="h2T_local")
        dT_local = dpool.tile([128, sp], F32, tag="dT_local")
        ar_in = dpool.tile([B, 257], F32, tag="ar_in")
        ar_out = dpool.tile([B, 257], F32, tag="ar_out", addr_space="Shared")

        # ---- generic GraphConv layer (F_out = 128) ------------------------
        def g_layer(lname, fin, gfin, tbl, idx_a, idx_b, ca, cb, dstf, ewf,
                    prevT_dram, wr, wroot, br, outT_dram, out_bf, out_bf_col,
                    npair=4):
            c = ca + cb
            for bp in range(0, n_blk, npair):
                np_ = min(npair, n_blk - bp)
                tot = np_ * c
                gath = gpool.tile([128, tot * gfin], BF16, tag="gath")
                for (qn, (idx, cc, off, th)) in enumerate((
                    (idx_a, ca, 0, tbl[0]),
                    (idx_b, cb, np_ * ca, tbl[1]),
                )):
                    span = np_ * cc
                    lo = span // 2
                    for (sq, c0, c1) in ((qn, 0, lo), (qn + 2, lo, span)):
                        if c1 == c0:
                            continue
                        nc.gpsimd.dma_gather(
                            out_ap=gath[:, (off + c0) * gfin:
                                        (off + c1) * gfin].rearrange(
                                "p (c f) -> p c f", c=c1 - c0),
                            in_ap=th,
                            idxs_ap=idx[:, bp * cc * 8 + c0 * 8:
                                        bp * cc * 8 + c1 * 8],
                            num_idxs=(c1 - c0) * 128,
                            num_idxs_reg=(c1 - c0) * 128,
                            elem_size=gfin,
                            single_packet=False,
                            queue_num=sq,
                        )
                gmm = gath

                wsel = wpool.tile([128, np_ * c * 128], BF16, tag="wsel")
                build_wsel(wsel, dstf, ewf, bp * c, np_ * c)

                for r in range(np_):
                    b = bp + r

                    def gpos(j, r=r):
                        if j < ca:
                            return r * ca + j
                        return np_ * ca + r * cb + (j - ca)

                    agg_ps = pagg.tile([fin, 128], F32, tag="agg_ps")
                    for j in range(c):
                        g0 = gpos(j) * gfin
                        nc.tensor.matmul(
                            out=agg_ps[:],
                            lhsT=gmm[:, g0:g0 + fin],
                            rhs=wsel[:, (r * c + j) * 128:
                                     (r * c + j + 1) * 128],
                            start=(j == 0), stop=(j == c - 1))
                    agg = spool.tile([fin, 128], F32, tag="agg_sb")
                    nc.scalar.copy(agg[:], agg_ps[:])

                    prevT = spool.tile([fin, 128], F32, tag="prevT")
                    nc.scalar.dma_start(
                        out=prevT[:],
                        in_=prevT_dram[:, b * 128:(b + 1) * 128])

                    o_ps = ppool2.tile([128, 128], F32, tag="o")
                    nc.tensor.matmul(out=o_ps[:], lhsT=wr[0][:], rhs=agg[:],
                                     start=True, stop=False)
                    nc.tensor.matmul(out=o_ps[:], lhsT=wroot[0][:],
                                     rhs=prevT[:], start=False, stop=True)
                    oT = spool.tile([128, 128], F32, tag="oT_sb")
                    nc.scalar.activation(out=oT[:], in_=o_ps[:], func=AF.Relu,
                                         bias=br[0][:, :1], scale=1.0)
                    nc.sync.dma_start(
                        out=outT_dram[:, b * 128:(b + 1) * 128], in_=oT[:])
                    # node-major bf16 copy for the next gather table
                    nm_ps = ppool.tile([128, 128], F32, tag="tp")
                    nc.tensor.transpose(out=nm_ps[:], in_=oT[:],
                                        identity=ident[:])
                    obf = spool.tile([128, 128], out_bf.dtype,
                                     tag="obf_sb")
                    nc.scalar.copy(obf[:], nm_ps[:])
                    nc.sync.dma_start(
                        out=out_bf[b * 128:(b + 1) * 128,
                                   out_bf_col:out_bf_col + 128],
                        in_=obf[:])

        # ---- g1 ------------------------------------------------------------
        g_layer("g1", 128, 128, (x_pad[0:half, :], x_pad[half:npad, :]),
                ev_ia, ev_ib, CA, CB, ev_d, ev_w,
                x_localT, W["g1_Wr"], W["g1_Wroot"], W["g1_br"],
                h1T_local, h1b_local, 0)
        nc.gpsimd.collective_compute(
            "AllGather", OP.bypass, replica_groups=[list(range(NC))],
            ins=[h1b_local.opt()], outs=[h1_full.opt()])

        # ---- d1 (independent of h1; overlaps the AllGather) ---------------
        g_layer("d1", 64, 128, (dur_pad[0:half, :], dur_pad[half:npad, :]),
                du_ia, du_ib, CDA, CDB, du_d, du_w,
                dur_localT, W["d1_Wr"], W["d1_Wroot"], W["d1_br"],
                dT_local, hd_local, 128)

        # ---- g2 ------------------------------------------------------------
        g_layer("g2", 128, 128, (h1_full[0:half, :], h1_full[half:npad, :]),
                ev_ia, ev_ib, CA, CB, ev_d, ev_w,
                h1T_local, W["g2_Wr"], W["g2_Wroot"], W["g2_br"],
                h2T_local, hd_local, 0)

        nc.gpsimd.collective_compute(
            "AllGather", OP.bypass, replica_groups=[list(range(NC))],
            ins=[hd_local.opt()], outs=[hd_full.opt()])

        def transpose_2(src, tag):
            ts = []
            for kh in range(2):
                t_ps = ppool.tile([128, B], F32, tag="tp")
                nc.tensor.transpose(out=t_ps[:],
                                    in_=src[:, kh * 128:(kh + 1) * 128],
                                    identity=ident[:B, :B])
                t_sb = spool.tile([128, B], F32, tag=f"{tag}_sb{kh}")
                nc.vector.tensor_copy(t_sb[:], t_ps[:])
                ts.append(t_sb)
            return ts

        def mlp(rhss, wname, bname, act, out_halves, tag):
            outs = []
            for o in range(out_halves):
                osl = slice(o * 128, (o + 1) * 128)
                ps = ppool2.tile([128, B], F32, tag="o")
                for si, r in enumerate(rhss):
                    nc.tensor.matmul(out=ps[:], lhsT=W[wname][si][:, osl],
                                     rhs=r[:], start=(si == 0),
                                     stop=(si == len(rhss) - 1))
                t = spool.tile([128, B], F32, tag=f"{tag}_sb{o}")
                nc.scalar.activation(out=t[:], in_=ps[:], func=act,
                                     bias=W[bname][o][:, :1], scale=1.0)
                outs.append(t)
            return outs

        # seq-feature MLP branch is independent of the GNN --
        # compute it here so it overlaps gather-phase idle time
        # instead of sitting in the post-gather tail.
        seq = spool.tile([B, 256], F32, tag="seq")
        nc.sync.dma_start(out=seq[:], in_=seq_in[:])
        seqT = transpose_2(seq, "seqT")

        s1T = mlp(seqT, "fc1_W", "fc1_b", AF.Relu, 2, "s1")
        sT = mlp(s1T, "fc2_W", "fc2_b", AF.Relu, 1, "s2")

        # ---- c1 + pooling --------------------------------------------------
        pooled = cpool.tile([B, 257], F32, tag="pooled")
        nc.vector.memset(pooled[:], 0.0)
        tbl = (hd_full[0:half, :], hd_full[half:npad, :])
        for bp in range(0, n_blk, 2):
            np_ = min(2, n_blk - bp)
            ctot = np_ * C
            gath = gpool.tile([128, ctot * 256], F8, tag="gath")
            for (qn, (idx, cc, off, th)) in enumerate((
                (ev_ia, CA, 0, tbl[0]),
                (ev_ib, CB, np_ * CA, tbl[1]),
            )):
                span = np_ * cc
                lo = span // 2
                for (sq, c0, c1) in ((qn, 0, lo), (qn + 2, lo, span)):
                    if c1 == c0:
                        continue
                    nc.gpsimd.dma_gather(
                        out_ap=gath[:, (off + c0) * 256:
                                    (off + c1) * 256].rearrange(
                            "p (c f) -> p c f", c=c1 - c0),
                        in_ap=th,
                        idxs_ap=idx[:, bp * cc * 8 + c0 * 8:
                                    bp * cc * 8 + c1 * 8],
                        num_idxs=(c1 - c0) * 128,
                        num_idxs_reg=(c1 - c0) * 128,
                        elem_size=256,
                        single_packet=False,
                        queue_num=sq,
                    )
            wsel = wpool.tile([128, np_ * C * 128], F8, tag="wsel")
            nc.scalar.dma_start(
                out=wsel[:, :np_ * C * 128],
                in_=ev_wsel8[:, bp * C * 128:(bp * C + np_ * C) * 128])

            for r in range(np_):
                b = bp + r

                def gpos_c1(j, r=r, np_=np_):
                    if j < CA:
                        return r * CA + j
                    return np_ * CA + r * CB + (j - CA)

                agg_f_ps = pacc.tile([128, 128], F32, tag="agg_f")
                agg_d_ps = pacc.tile([128, 128], F32, tag="agg_d")
                for j in range(C):
                    g0 = gpos_c1(j) * 256
                    ws = wsel[:, (r * C + j) * 128:(r * C + j + 1) * 128]
                    nc.tensor.matmul(
                        out=agg_f_ps[:], lhsT=gath[:, g0:g0 + 128],
                        rhs=ws, start=(j == 0), stop=(j == C - 1))
                    nc.tensor.matmul(
                        out=agg_d_ps[:], lhsT=gath[:, g0 + 128:g0 + 256],
                        rhs=ws, start=(j == 0), stop=(j == C - 1))
                agg_f = spool.tile([128, 128], F32, tag="aggc_f")
                nc.scalar.copy(agg_f[:], agg_f_ps[:])
                agg_d = spool.tile([128, 128], F32, tag="aggc_d")
                nc.scalar.copy(agg_d[:], agg_d_ps[:])
                aggs = (agg_f, agg_d)

                xcT = []
                for kh, src_dram in ((0, h2T_local), (1, dT_local)):
                    t_sb = spool.tile([128, 128], F32, tag=f"xcT_sb{kh}")
                    nc.scalar.dma_start(
                        out=t_sb[:],
                        in_=src_dram[:, b * 128:(b + 1) * 128])
                    xcT.append(t_sb)

                # node-major y: out[dst, fout] accumulated in one PSUM tile;
                # bias added via a rank-1 ones-row matmul, so relu needs no
                # per-partition bias
                y_ps = ppool2.tile([128, 256], F32, tag="o")
                nc.tensor.matmul(out=y_ps[:], lhsT=ones_row[:1, :],
                                 rhs=W["bias_c_row"][0][:1, :],
                                 start=True, stop=False)
                for kh in range(2):
                    nc.tensor.matmul(out=y_ps[:], lhsT=aggs[kh][:],
                                     rhs=W["c1_Wr"][kh][:], start=False,
                                     stop=False)
                    nc.tensor.matmul(out=y_ps[:], lhsT=xcT[kh][:],
                                     rhs=W_rs[kh][:], start=False,
                                     stop=(kh == 1))
                y_nm = spool.tile([128, 272], F32, tag="y_nm")
                nc.vector.memset(y_nm[:, 256:257], 1.0)
                nc.scalar.activation(out=y_nm[:, :256], in_=y_ps[:],
                                     func=AF.Relu)
                pool_ps = ppool2.tile([B, 257], F32, tag="o")
                nc.tensor.matmul(out=pool_ps[:],
                                 lhsT=ssel[:, b * B:(b + 1) * B],
                                 rhs=y_nm[:, :257], start=True, stop=True)
                nc.vector.tensor_add(out=pooled[:], in0=pooled[:],
                                     in1=pool_ps[:])

        # ---- AllReduce pooled sums + counts -------------------------------
        nc.sync.dma_start(out=ar_in[:], in_=pooled[:])
        nc.gpsimd.collective_compute(
            "AllReduce", OP.add, replica_groups=[list(range(NC))],
            ins=[ar_in.opt()], outs=[ar_out.opt()])

        # ---- head (replicated on every core) ------------------------------
        pl = spool.tile([B, 257], F32, tag="pl")
        nc.sync.dma_start(out=pl[:], in_=ar_out[:])
        cnt = spool.tile([B, 1], F32, tag="cnt")
        nc.vector.tensor_scalar(out=cnt[:], in0=pl[:, 256:257], scalar1=1.0,
                                scalar2=None, op0=OP.max)
        rec = spool.tile([B, 1], F32, tag="rec")
        nc.vector.reciprocal(out=rec[:], in_=cnt[:])
        emb = spool.tile([B, 256], F32, tag="emb")
        nc.vector.tensor_tensor(out=emb[:], in0=pl[:, :256],
                                in1=rec[:, :1].to_broadcast([B, 256]),
                                op=OP.mult)

        embT = transpose_2(emb, "embT")

        hT = mlp(embT + sT, "fcc_W", "fcc_b", AF.Relu, 2, "hc")

        lg_ps = ppool2.tile([B, 16], F32, tag="o")
        for o in range(2):
            nc.tensor.matmul(out=lg_ps[:], lhsT=hT[o][:], rhs=W["cls_W"][o][:],
                             start=(o == 0), stop=(o == 1))
        logits = spool.tile([B, 16], F32, tag="logits")
        nc.vector.tensor_tensor(out=logits[:], in0=lg_ps[:],
                                in1=W["cls_b_rep"][0][:], op=OP.add)
        rmax = spool.tile([B, 1], F32, tag="rmax")
        nc.vector.tensor_reduce(out=rmax[:], in_=logits[:],
                                axis=mybir.AxisListType.X, op=OP.max)
        tshift = spool.tile([B, 16], F32, tag="tshift")
        nc.vector.tensor_scalar(out=tshift[:], in0=logits[:],
                                scalar1=rmax[:, :1], scalar2=None,
                                op0=OP.subtract)
        ex = spool.tile([B, 16], F32, tag="ex")
        nc.scalar.activation(out=ex[:], in_=tshift[:], func=AF.Exp)
        esum = spool.tile([B, 1], F32, tag="esum")
        nc.vector.tensor_reduce(out=esum[:], in_=ex[:],
                                axis=mybir.AxisListType.X, op=OP.add)
        lsum = spool.tile([B, 1], F32, tag="lsum")
        nc.scalar.activation(out=lsum[:], in_=esum[:], func=AF.Ln)
        res = spool.tile([B, 16], F32, tag="res")
        nc.vector.tensor_scalar(out=res[:], in0=tshift[:],
                                scalar1=lsum[:, :1], scalar2=None,
                                op0=OP.subtract)
        nc.sync.dma_start(out=out_ext[:], in_=res[:])

    nc.compile()
    return nc


# --------------------------------------------------------------------------
# Host orchestration
# --------------------------------------------------------------------------

def make_in_maps(inputs, cfg):
    import ml_dtypes
    x = np.asarray(inputs["x"], np.float32)
    # Reference masks x at -1.0 sentinels (and the post-layer masks are
    # no-ops given relu(-1.0) == 0), so pre-mask on host once.
    x = np.where(x == -1.0, 0.0, x)
    dur_x = np.asarray(inputs["dur_x"], np.float32)
    batch = np.asarray(inputs["batch"], np.int64)

    ev_planes, CA, CB = prep_edges(inputs["edge_index"], inputs["edge_attr"],
                                   cfg)
    du_planes, CDA, CDB = prep_edges(inputs["dur_edge_index"],
                                     inputs["dur_edge_attr"], cfg)

    x_pad_f32 = _pad_nodes(x, cfg)
    x_pad = x_pad_f32.astype(ml_dtypes.bfloat16)
    dur_padded = _pad_nodes(dur_x, cfg)
    # bf16 dur table padded to 128 features (256B rows for the gather)
    dur_pad_bf16 = np.zeros((dur_padded.shape[0], 128), ml_dtypes.bfloat16)
    dur_pad_bf16[:, :64] = dur_padded.astype(ml_dtypes.bfloat16)

    n_blk = cfg["SHARD_PAD"] // 128
    B = cfg["B"]
    bias_c = (np.asarray(inputs["c1_br"], np.float32)
              + np.asarray(inputs["skip_b"], np.float32))

    def col(v):
        return np.ascontiguousarray(
            np.asarray(v, np.float32).reshape(-1, 1))

    weights = dict(
        g1_Wr=inputs["g1_Wr"], g1_br=col(inputs["g1_br"]),
        g1_Wroot=inputs["g1_Wroot"],
        g2_Wr=inputs["g2_Wr"], g2_br=col(inputs["g2_br"]),
        g2_Wroot=inputs["g2_Wroot"],
        d1_Wr=inputs["d1_Wr"], d1_br=col(inputs["d1_br"]),
        d1_Wroot=inputs["d1_Wroot"],
        c1_Wr=inputs["c1_Wr"], c1_Wroot=inputs["c1_Wroot"],
        skip_W=inputs["skip_W"],
        bias_c_row=np.asarray(bias_c, np.float32).reshape(1, -1),
        fc1_W=inputs["fc1_W"], fc1_b=col(inputs["fc1_b"]),
        fc2_W=inputs["fc2_W"], fc2_b=col(inputs["fc2_b"]),
        fcc_W=inputs["fcc_W"], fcc_b=col(inputs["fcc_b"]),
        cls_W=inputs["cls_W"],
        cls_b_rep=np.tile(np.asarray(inputs["cls_b"], np.float32)[None, :],
                          (B, 1)),
        seq_features=inputs["seq_features"],
    )
    weights = {k: np.ascontiguousarray(np.asarray(v, np.float32))
               for k, v in weights.items()}

    in_maps = []
    for k in range(NC):
        sp = cfg["SHARD_PAD"]
        # graph-membership one-hot [128 node-in-block, n_blk * B]
        bfr_flat = np.full(sp, -1, np.int64)
        bfr_flat[:cfg["SHARD"]] = batch[k * cfg["SHARD"]:(k + 1) * cfg["SHARD"]]
        ssel = np.zeros((n_blk, 128, B), np.float32)
        bb = bfr_flat.reshape(n_blk, 128)
        blk_i, pos_i = np.nonzero(bb >= 0)
        ssel[blk_i, pos_i, bb[blk_i, pos_i]] = 1.0
        ssel = np.ascontiguousarray(
            ssel.transpose(1, 0, 2).reshape(128, n_blk * B))

        m = dict(
            x_pad=x_pad,
            x_localT=np.ascontiguousarray(
                x_pad_f32[k * sp:(k + 1) * sp].T),
            dur_pad=dur_pad_bf16,
            dur_localT=np.ascontiguousarray(
                dur_padded[k * sp:(k + 1) * sp].T),
            ev_idx_a=ev_planes[k]["idx_a"], ev_idx_b=ev_planes[k]["idx_b"],
            ev_dstf=ev_planes[k]["dstf"], ev_ewf=ev_planes[k]["ewf"],
            ev_wsel8=ev_planes[k]["wsel8"],
            du_idx_a=du_planes[k]["idx_a"], du_idx_b=du_planes[k]["idx_b"],
            du_dstf=du_planes[k]["dstf"], du_ewf=du_planes[k]["ewf"],
            ssel=ssel,
            **weights,
        )
        in_maps.append(m)
    return in_maps, (CA, CB, CDA, CDB)


_LAST_RESULT = None


def kernel(**inputs) -> np.ndarray:
    global _LAST_RESULT
    cfg = dict(REAL)
    cfg["N"] = inputs["x"].shape[0]
    cfg["B"] = inputs["seq_features"].shape[0]
    in_maps, (CA, CB, CDA, CDB) = make_in_maps(inputs, cfg)
    nc = build_program(cfg, CA, CB, CDA, CDB)
    from concourse.bass_utils import run_bass_kernel_spmd
    res = run_bass_kernel_spmd(nc, in_maps, list(range(NC)))
    _LAST_RESULT = res
    return np.asarray(res.results[0]["out"], np.float32)

